# revision 1
# baseline (speedup 1.0000x reference)
"""Trainium2 Bass kernel for nn_AttentionModelCharLevel.

Model (per reference): visual linear -> char-encoder LSTM -> linear+relu ->
cosine attention (softmax over batch dim) -> char-decoder LSTM -> per-sample
mean NLL over L steps.

Sharding: data-parallel over batch B=4096 across 8 cores (512 rows each).
The [B,B] attention needs every core to see all normalized visual rows, so
each core computes its vn shard (+transpose), AllGathers both, and streams
the gathered rows back through SBUF during the attention phase.

Key device-side conventions:
- High-count matmuls (gates/logits/vis/attention) use bf16 operands with
  fp32 PSUM accumulation (full PE rate + hidden fast-weight-load); the
  small ones-matmul reductions stay float32r.
- sigmoid(z) = 0.5*tanh(z/2)+0.5 so the whole LSTM scan only needs Tanh
  (exp_and_others ACT table set, which also has Exp for the decoder
  softmax partition-sums) -- no per-step table switches.
- Hidden state is stored doubled (Ht = 2h, St = 2c). The 0.5 factors fold
  into W_hh, W_enc, W_out host-side; the LSTM cell update then becomes four
  fused scalar_tensor_tensor ops per 128-row chunk:
      m2 = (tanh_i + 1) * tanh_g
      m1 = (tanh_f + 1) * S
      S' = 0.5*m1 + m2
      H' = (tanh_o + 1) * tanh(0.5*S')
- Softmax over the batch dim reduces to exp() only: sims are cosine
  similarities in [-1,1] (no max subtraction needed) and the softmax
  denominator is a positive per-column scale that the final row
  normalization of h cancels.
- Decoder log-softmax: logits kept in [V, B] layout; Z = ones^T exp(logits)
  and the target logit via a one-hot mask built from an iota column --
  both reduce over partitions with K=128 ones-matmuls. ln() deferred to a
  single post-pass (natural_log table set).
"""
import os
import sys

sys.path.insert(0, '/opt/trn_rl_repo')

import numpy as np

B_FULL = 4096
NCORES = 8
B = B_FULL // NCORES          # 512 rows per core
H = 512
G = 4 * H                     # 2048
E = 50
L = 16
V = 128
VIS = 2048
HK = H // 128                 # 4 chunks of the hidden dim
GK = G // 128                 # 16 gate chunks
VISK = VIS // 128             # 16 chunks of the visual dim
BK = B // 128                 # 4 batch chunks per core
VCHUNKS = B_FULL // 128       # 32 chunks of the full batch

_CACHE = {}


def _build():
    import concourse.bass as bass
    import concourse.tile as tile
    import concourse.mybir as mybir
    from concourse import bacc
    from concourse.masks import make_identity
    from contextlib import ExitStack

    dt = mybir.dt
    AF = mybir.ActivationFunctionType
    ALU = mybir.AluOpType
    f32 = dt.float32
    f32r = dt.float32r
    bf16d = dt.bfloat16
    LN2 = float(np.log(2.0))

    AP = bass.AP
    nc = bacc.Bacc("TRN2", target_bir_lowering=False, debug=False,
                   num_devices=NCORES)

    # ---- DRAM I/O ----
    visT_d = nc.dram_tensor("visT", [VISK, 128, B], bf16d, kind="ExternalInput").ap()
    WvisT_d = nc.dram_tensor("WvisT", [VISK, 128, H], bf16d, kind="ExternalInput").ap()
    WihT_d = nc.dram_tensor("WihT", [E, G], bf16d, kind="ExternalInput").ap()
    WhhT_d = nc.dram_tensor("WhhT", [HK, 128, G], bf16d, kind="ExternalInput").ap()
    biasg_d = nc.dram_tensor("biasg", [128, GK], f32, kind="ExternalInput").ap()
    encx_d = nc.dram_tensor("encx", [L, E, B], bf16d, kind="ExternalInput").ap()
    decx_d = nc.dram_tensor("decx", [L, E, B], bf16d, kind="ExternalInput").ap()
    WencT_d = nc.dram_tensor("WencT", [HK, 128, H], bf16d, kind="ExternalInput").ap()
    benc_d = nc.dram_tensor("benc", [128, HK], f32, kind="ExternalInput").ap()
    WoutT_d = nc.dram_tensor("WoutT", [HK, 128, V], bf16d, kind="ExternalInput").ap()
    tgt_d = nc.dram_tensor("tgt", [L, B], f32, kind="ExternalInput").ap()
    iota_d = nc.dram_tensor("iota128", [128, 1], f32, kind="ExternalInput").ap()
    ones_d = nc.dram_tensor("ones128", [128, 1], f32r, kind="ExternalInput").ap()
    out_d = nc.dram_tensor("loss", [1, B], f32, kind="ExternalOutput").ap()

    with tile.TileContext(nc) as tc, ExitStack() as top:
        wpool = top.enter_context(tc.tile_pool(name="w", bufs=1))
        spool = top.enter_context(tc.tile_pool(name="state", bufs=2))
        dram = top.enter_context(tc.tile_pool(name="dram", bufs=1, space="DRAM"))

        # ---- persistent weights / constants ----
        WhhT = [wpool.tile([128, G], bf16d, tag=f"whh{k}", name=f"whh{k}") for k in range(HK)]
        for k in range(HK):
            nc.sync.dma_start(WhhT[k][:], WhhT_d[k])
        WihT2 = wpool.tile([114, G], bf16d, tag="wih", name="wih")
        nc.sync.dma_start(WihT2[:E, :], WihT_d)
        nc.sync.dma_start(WihT2[64:64 + E, :], WihT_d)
        biasg = wpool.tile([128, GK], f32, tag="biasg", name="biasg")
        nc.sync.dma_start(biasg[:], biasg_d)
        WencT = [wpool.tile([128, H], bf16d, tag=f"wenc{k}", name=f"wenc{k}") for k in range(HK)]
        for k in range(HK):
            nc.sync.dma_start(WencT[k][:], WencT_d[k])
        benc = wpool.tile([128, HK], f32, tag="benc", name="benc")
        nc.sync.dma_start(benc[:], benc_d)
        WoutT = wpool.tile([128, HK, V], bf16d, tag="wout", name="wout")
        nc.sync.dma_start(WoutT[:], WoutT_d.rearrange("k p v -> p k v"))
        iota_c = wpool.tile([128, 1], f32, tag="iota", name="iota")
        nc.sync.dma_start(iota_c[:], iota_d)
        ones_col = wpool.tile([128, 1], f32r, tag="ones_col", name="ones_col")
        nc.sync.dma_start(ones_col[:], ones_d)
        ones_row = wpool.tile([1, 128], f32r, tag="ones_row", name="ones_row")
        nc.sync.dma_start(ones_row[:], ones_d.rearrange("p one -> one p"))
        ones16 = wpool.tile([L, 1], f32r, tag="ones16", name="ones16")
        nc.sync.dma_start(ones16[:], ones_d[:L])
        ident = wpool.tile([128, 128], bf16d, tag="ident", name="ident")
        make_identity(nc, ident[:])
        ln2_t = wpool.tile([1, 1], f32, tag="ln2", name="ln2")
        nc.vector.memset(ln2_t[:], LN2)

        # AllGather buffers: vn blocks and vnT blocks (both bf16)
        ag_in = dram.tile([BK, 128, B], bf16d, name="ag_in")
        ag_out = dram.tile([NCORES, BK, 128, B], bf16d, addr_space="Shared", name="ag_out")
        agt_in = dram.tile([HK, 128, B], bf16d, name="agt_in")
        agt_out = dram.tile([NCORES, HK, 128, B], bf16d, addr_space="Shared", name="agt_out")

        # decoder per-step Z and target-logit rows
        zpool = top.enter_context(tc.tile_pool(name="zp", bufs=1))
        Zs = zpool.tile([L, B], f32, tag="Zs", name="Zs")
        lts = zpool.tile([L, B], f32r, tag="lts", name="lts")

        # ======== Phase 1: visual linear + row-normalize + transpose + AG ====
        with ExitStack() as ph:
            vsb = ph.enter_context(tc.tile_pool(name="vsb", bufs=3))
            vps = ph.enter_context(tc.tile_pool(name="vps", bufs=1, space="PSUM"))
            tps = ph.enter_context(tc.tile_pool(name="tps", bufs=2, space="PSUM"))
            vnpool = ph.enter_context(tc.tile_pool(name="vnp", bufs=1))

            v_ps = [vps.tile([128, H], f32, tag=f"vps{b}", name=f"vps{b}") for b in range(BK)]
            for ki in range(VISK):
                vis_t = vsb.tile([128, B], bf16d, tag="vis", name="vis")
                nc.sync.dma_start(vis_t[:], visT_d[ki])
                wv_t = vsb.tile([128, H], bf16d, tag="wvis", name="wvis")
                nc.sync.dma_start(wv_t[:], WvisT_d[ki])
                for b in range(BK):
                    nc.tensor.matmul(v_ps[b][:], vis_t[:, b * 128:(b + 1) * 128],
                                     wv_t[:], start=(ki == 0), stop=(ki == VISK - 1))
            vn = []
            for b in range(BK):
                sq = vsb.tile([128, H], f32, tag="vsq", name="vsq")
                s_col = vsb.tile([128, 1], f32, tag="vscol", name="vscol")
                nc.scalar.activation(sq[:], v_ps[b][:], AF.Square,
                                     accum_out=s_col[:])
                lnc_ = vsb.tile([128, 1], f32, tag="vln", name="vln")
                nc.scalar.activation(lnc_[:], s_col[:], AF.Ln)
                rs = vsb.tile([128, 1], f32, tag="vrs", name="vrs")
                nc.scalar.activation(rs[:], lnc_[:], AF.Exp, scale=-0.5)
                vn_b = vnpool.tile([128, H], bf16d, tag=f"vn{b}", name=f"vn{b}")
                nc.vector.tensor_scalar(vn_b[:], v_ps[b][:], rs[:], None, ALU.mult)
                vn.append(vn_b)
            # transpose vn -> vnT (16 128x128 blocks, bf16 for the sims lhsT)
            vnT = [vnpool.tile([128, B], bf16d, tag=f"vnT{h}", name=f"vnT{h}") for h in range(HK)]
            for b in range(BK):
                for h in range(HK):
                    t_ps = tps.tile([128, 128], bf16d, tag="tr", name="tr")
                    nc.tensor.transpose(
                        t_ps[:], vn[b][:, h * 128:(h + 1) * 128], ident[:])
                    nc.vector.tensor_copy(vnT[h][:, b * 128:(b + 1) * 128], t_ps[:])
            for b in range(BK):
                nc.sync.dma_start(ag_in[b], vn[b][:])
            for h in range(HK):
                nc.sync.dma_start(agt_in[h], vnT[h][:])
            nc.gpsimd.collective_compute(
                "AllGather", mybir.AluOpType.bypass,
                replica_groups=[list(range(NCORES))],
                ins=[ag_in[:]], outs=[ag_out[:]],
            )
            nc.gpsimd.collective_compute(
                "AllGather", mybir.AluOpType.bypass,
                replica_groups=[list(range(NCORES))],
                ins=[agt_in[:]], outs=[agt_out[:]],
            )

        # staged full vnT (bf16) for the attention sims lhsT; filled by
        # gpsimd DMAs that wait on the collective and run during the encoder
        vnT_all = [wpool.tile([128, B_FULL], bf16d, tag=f"vnTall{k}", name=f"vnTall{k}")
                   for k in range(HK)]
        for k in range(HK):
            for r in range(NCORES):
                nc.gpsimd.dma_start(vnT_all[k][:, r * B:(r + 1) * B], agt_out[r, k])

        # ======== LSTM scan helper ========
        xsb = top.enter_context(tc.tile_pool(name="xsb", bufs=3))
        gsb = top.enter_context(tc.tile_pool(name="gsb", bufs=2))
        msb = top.enter_context(tc.tile_pool(name="msb", bufs=2))

        def lstm_step(gps, x_src, Hp, Sp, max_open_pairs=3):
            """One LSTM step, software-pipelined over PSUM banks.

            Gate chunks are processed in pairs (i,f) / (g,o) per hidden
            chunk j. Opening a pair issues both x-matmuls back-to-back into
            disjoint PE row groups (K=50 at partitions 0 and 64, so they run
            concurrently) plus the k0..k2 recurrence matmuls; the k3 matmul
            and the tanh are deferred until the pair's PSUM banks must be
            recycled. This keeps the PE streaming into the next step while
            the previous step's last H-chunk chain drains on ACT/DVE.
            """
            xt = xsb.tile([114, B], bf16d, tag="xt", name="xt")
            nc.sync.dma_start(xt[:E, :], x_src)
            nc.sync.dma_start(xt[64:64 + E, :], x_src)
            Hn, Sn = [None] * HK, [None] * HK
            gt = {}

            def open_pair(j, gates):
                ps_pair = []
                for idx, gate in enumerate(gates):
                    c = gate * 4 + j
                    ps = gps.tile([128, B], f32, tag="gps", name="gps")
                    r0 = 0 if idx == 0 else 64
                    nc.tensor.matmul(ps[:], WihT2[r0:r0 + E, c * 128:(c + 1) * 128],
                                     xt[r0:r0 + E, :], start=True, stop=False)
                    ps_pair.append(ps)
                for gate, ps in zip(gates, ps_pair):
                    c = gate * 4 + j
                    for ki in range(HK - 1):
                        nc.tensor.matmul(ps[:],
                                         WhhT[ki][:, c * 128:(c + 1) * 128],
                                         Hp[ki][:], start=False, stop=False)
                return (j, gates, ps_pair)

            def close_pair(entry):
                j, gates, ps_pair = entry
                for gate, ps in zip(gates, ps_pair):
                    c = gate * 4 + j
                    nc.tensor.matmul(ps[:],
                                     WhhT[HK - 1][:, c * 128:(c + 1) * 128],
                                     Hp[HK - 1][:], start=False, stop=True)
                for gate, ps in zip(gates, ps_pair):
                    c = gate * 4 + j
                    g_t = gsb.tile([128, B], f32, tag=f"g{gate}", name=f"g{gate}")
                    nc.scalar.activation(
                        g_t[:], ps[:], AF.Tanh,
                        bias=biasg[:, c:c + 1],
                        scale=(1.0 if gate == 2 else 0.5))
                    gt[(j, gate)] = g_t
                if (j, 3) in gt:      # chunk complete -> state chain
                    m2 = msb.tile([128, B], f32, tag="m2", name="m2")
                    nc.vector.scalar_tensor_tensor(m2[:], gt[(j, 0)][:], 1.0,
                                                   gt[(j, 2)][:], ALU.add, ALU.mult)
                    m1 = msb.tile([128, B], f32, tag="m1", name="m1")
                    nc.vector.scalar_tensor_tensor(m1[:], gt[(j, 1)][:], 1.0,
                                                   Sp[j][:], ALU.add, ALU.mult)
                    Sn[j] = spool.tile([128, B], f32, tag=f"S{j}", name=f"S{j}")
                    nc.vector.scalar_tensor_tensor(Sn[j][:], m1[:], 0.5, m2[:],
                                                   ALU.mult, ALU.add)
                    th = msb.tile([128, B], f32, tag="th", name="th")
                    nc.scalar.activation(th[:], Sn[j][:], AF.Tanh, scale=0.5)
                    Hn[j] = spool.tile([128, B], bf16d, tag=f"H{j}", name=f"H{j}")
                    nc.vector.scalar_tensor_tensor(Hn[j][:], gt[(j, 3)][:], 1.0,
                                                   th[:], ALU.add, ALU.mult)

            from collections import deque
            open_q = deque()
            for j in range(HK):
                for gates in ((0, 1), (2, 3)):
                    if len(open_q) >= max_open_pairs:
                        close_pair(open_q.popleft())
                    open_q.append(open_pair(j, gates))
            while open_q:
                close_pair(open_q.popleft())
            return Hn, Sn

        # ======== Phase 2: encoder ========
        Hp = [spool.tile([128, B], bf16d, tag=f"H{j}", name=f"H{j}") for j in range(HK)]
        Sp = [spool.tile([128, B], f32, tag=f"S{j}", name=f"S{j}") for j in range(HK)]
        init02 = xsb.tile([128, B], f32, tag="init02", name="init02", bufs=1)
        nc.vector.memset(init02[:], 0.2)
        for j in range(HK):
            nc.vector.tensor_copy(Hp[j][:], init02[:])
            nc.vector.memset(Sp[j][:], 0.2)
        with tc.tile_pool(name="gpse", bufs=6, space="PSUM") as gps_e:
            for s in range(L):
                Hp, Sp = lstm_step(gps_e, encx_d[s], Hp, Sp)
        Henc = Hp

        # ======== Phase 3: t path + attention ========
        H0 = [None] * HK
        S0 = [None] * HK
        with ExitStack() as ph:
            asb = ph.enter_context(tc.tile_pool(name="asb", bufs=2))
            aps = ph.enter_context(tc.tile_pool(name="aps", bufs=1, space="PSUM"))
            sps_pool = ph.enter_context(tc.tile_pool(name="sps", bufs=2, space="PSUM"))
            vstr = ph.enter_context(tc.tile_pool(name="vstr", bufs=3))

            # t = relu(Wenc' @ Henc + benc), column-normalized
            tr = []
            s_ps = aps.tile([1, B], f32, tag="tsum", name="tsum")
            for mi in range(HK):
                t_ps = sps_pool.tile([128, B], f32, tag="sims", name="sims")
                for ki in range(HK):
                    nc.tensor.matmul(t_ps[:],
                                     WencT[ki][:, mi * 128:(mi + 1) * 128],
                                     Henc[ki][:], start=(ki == 0),
                                     stop=(ki == HK - 1))
                tr_mi = asb.tile([128, B], f32, tag=f"tr{mi}", name=f"tr{mi}", bufs=1)
                nc.scalar.activation(tr_mi[:], t_ps[:], AF.Relu,
                                     bias=benc[:, mi:mi + 1])
                tr.append(tr_mi)
                sq = asb.tile([128, B], f32r, tag="tsq", name="tsq")
                nc.scalar.activation(sq[:], tr_mi[:], AF.Square)
                nc.tensor.matmul(s_ps[:], ones_col[:], sq[:],
                                 start=(mi == 0), stop=(mi == HK - 1))
            lnr = asb.tile([1, B], f32, tag="tlnr", name="tlnr")
            nc.scalar.activation(lnr[:], s_ps[:], AF.Ln)
            rs_r = asb.tile([1, B], f32r, tag="trs", name="trs")
            nc.scalar.activation(rs_r[:], lnr[:], AF.Exp, scale=-0.5)
            bc_ps = aps.tile([128, B], f32, tag="tbc", name="tbc")
            nc.tensor.matmul(bc_ps[:], ones_row[:], rs_r[:], start=True, stop=True)
            tnT = []
            for mi in range(HK):
                tn_mi = asb.tile([128, B], bf16d, tag=f"tn{mi}", name=f"tn{mi}", bufs=1)
                nc.vector.tensor_tensor(tn_mi[:], tr[mi][:], bc_ps[:], ALU.mult)
                tnT.append(tn_mi)

            # attention: stream the gathered vn/vnT, E=exp(sims), accumulate h
            hu_ps = [aps.tile([128, B], f32, tag=f"hu{h}", name=f"hu{h}") for h in range(HK)]
            for i in range(VCHUNKS):
                r, b = divmod(i, BK)
                vn_i = vstr.tile([128, B], bf16d, tag="vni", name="vni", bufs=4)
                nc.sync.dma_start(vn_i[:], ag_out[r, b])
                sim_ps = sps_pool.tile([128, B], f32, tag="sims", name="sims")
                for k in range(HK):
                    nc.tensor.matmul(sim_ps[:],
                                     vnT_all[k][:, i * 128:(i + 1) * 128],
                                     tnT[k][:],
                                     start=(k == 0), stop=(k == HK - 1))
                E_i = vstr.tile([128, B], bf16d, tag="E", name="E")
                nc.scalar.activation(E_i[:], sim_ps[:], AF.Exp)
                for h in range(HK):
                    nc.tensor.matmul(hu_ps[h][:],
                                     vn_i[:, h * 128:(h + 1) * 128], E_i[:],
                                     start=(i == 0), stop=(i == VCHUNKS - 1))
            # normalize h (x2 for the doubled-state convention) -> decoder init
            s2_ps = aps.tile([1, B], f32, tag="tsum", name="tsum")
            for h in range(HK):
                sq = asb.tile([128, B], f32r, tag="husq", name="husq")
                nc.scalar.activation(sq[:], hu_ps[h][:], AF.Square)
                nc.tensor.matmul(s2_ps[:], ones_col[:], sq[:],
                                 start=(h == 0), stop=(h == HK - 1))
            lnr2 = asb.tile([1, B], f32, tag="hulnr", name="hulnr")
            nc.scalar.activation(lnr2[:], s2_ps[:], AF.Ln)
            rs2 = asb.tile([1, B], f32r, tag="hurs", name="hurs")
            nc.scalar.activation(rs2[:], lnr2[:], AF.Exp, scale=-0.5, bias=ln2_t[:])
            bc2_ps = aps.tile([128, B], f32, tag="tbc", name="tbc")
            nc.tensor.matmul(bc2_ps[:], ones_row[:], rs2[:], start=True, stop=True)
            bc2_sb = asb.tile([128, B], f32, tag="bc2sb", name="bc2sb", bufs=1)
            nc.vector.tensor_copy(bc2_sb[:], bc2_ps[:])
            for h in range(HK):
                H0[h] = spool.tile([128, B], bf16d, tag=f"H{h}", name=f"H{h}")
                nc.vector.tensor_tensor(H0[h][:], hu_ps[h][:], bc2_sb[:], ALU.mult)
                S0[h] = spool.tile([128, B], f32, tag=f"S{h}", name=f"S{h}")
                nc.vector.tensor_tensor(S0[h][:], hu_ps[h][:], bc2_sb[:], ALU.mult)

        # ======== Phase 4: decoder ========
        dsb = top.enter_context(tc.tile_pool(name="dsb", bufs=2))
        with ExitStack() as ph:
            gps_d = ph.enter_context(tc.tile_pool(name="gpsd", bufs=6, space="PSUM"))
            dps = ph.enter_context(tc.tile_pool(name="dps", bufs=1, space="PSUM"))
            zps_pool = ph.enter_context(tc.tile_pool(name="zpp", bufs=1, space="PSUM"))
            Hp, Sp = H0, S0
            for s in range(L):
                Hp, Sp = lstm_step(gps_d, decx_d[s], Hp, Sp)
                # logitsT [V, B]
                l_ps = dps.tile([128, B], f32, tag="lps", name="lps")
                for ki in range(HK):
                    nc.tensor.matmul(l_ps[:], WoutT[:, ki, :], Hp[ki][:],
                                     start=(ki == 0), stop=(ki == HK - 1))
                El = dsb.tile([128, B], f32r, tag="El", name="El")
                nc.scalar.activation(El[:], l_ps[:], AF.Exp)
                z_ps = zps_pool.tile([1, B], f32, tag="zlt", name="zlt")
                nc.tensor.matmul(z_ps[:], ones_col[:], El[:], start=True, stop=True)
                ztmp = dsb.tile([1, B], f32, tag="ztmp", name="ztmp")
                nc.vector.tensor_copy(ztmp[:], z_ps[:])
                nc.sync.dma_start(Zs[s:s + 1, :], ztmp[:])
                # target logit via iota==tgt mask
                tb = dsb.tile([128, B], f32, tag="tb", name="tb")
                tb_src = AP(tensor=tgt_d.tensor, offset=s * B,
                            ap=[[0, 128], [1, B]])
                nc.gpsimd.dma_start(tb[:], tb_src)
                mk = dsb.tile([128, B], f32r, tag="mk", name="mk")
                nc.vector.scalar_tensor_tensor(mk[:], tb[:], iota_c[:], l_ps[:],
                                               ALU.is_equal, ALU.mult)
                lt_ps = zps_pool.tile([1, B], f32, tag="zlt", name="zlt")
                nc.tensor.matmul(lt_ps[:], ones_col[:], mk[:], start=True, stop=True)
                lttmp = dsb.tile([1, B], f32r, tag="lttmp", name="lttmp")
                nc.vector.tensor_copy(lttmp[:], lt_ps[:])
                nc.sync.dma_start(lts[s:s + 1, :], lttmp[:])

            # ======== Phase 5: final loss ========
            lnZ = dsb.tile([L, B], f32r, tag="lnZ", name="lnZ")
            nc.scalar.activation(lnZ[:], Zs[:], AF.Ln)
            diff = dsb.tile([L, B], f32r, tag="diff", name="diff")
            nc.vector.tensor_tensor(diff[:], lnZ[:], lts[:], ALU.subtract)
            loss_ps = zps_pool.tile([1, B], f32, tag="zlt", name="zlt")
            nc.tensor.matmul(loss_ps[:], ones16[:], diff[:], start=True, stop=True)
            loss_sb = dsb.tile([1, B], f32, tag="losssb", name="losssb")
            nc.vector.tensor_scalar(loss_sb[:], loss_ps[:], 1.0 / L, None, ALU.mult)
            nc.sync.dma_start(out_d, loss_sb[:])

    nc.compile()
    return nc


def _prep_inputs(visual_input, text_input, emb, W_ih, W_hh, b_ih, b_hh,
                 W_enc, b_enc, W_out, W_vis):
    import ml_dtypes
    bf = ml_dtypes.bfloat16
    f = np.float32
    vis = np.asarray(visual_input, f)[:, 0, :]              # [4096, 2048]
    text = np.asarray(text_input)
    emb = np.asarray(emb, f)
    visT = np.ascontiguousarray(vis.T)                      # [2048, 4096]
    WvisT = np.ascontiguousarray(np.asarray(W_vis, f).T)    # [2048, 512]
    WihT = np.ascontiguousarray(np.asarray(W_ih, f).T)      # [50, 2048]
    WhhT = np.ascontiguousarray((0.5 * np.asarray(W_hh, f)).T)   # [512, 2048]
    WencT = np.ascontiguousarray((0.5 * np.asarray(W_enc, f)).T)  # [512, 512]
    WoutT = np.ascontiguousarray((0.5 * np.asarray(W_out, f)).T)  # [512, 128]
    b = np.asarray(b_ih, f) + np.asarray(b_hh, f)           # [2048]
    scale = np.ones(G, f) * 0.5
    scale[2 * H:3 * H] = 1.0                                # g-gate keeps full bias
    biasg = np.ascontiguousarray((b * scale).reshape(GK, 128).T)  # [128, 16]
    benc = np.ascontiguousarray(np.asarray(b_enc, f).reshape(HK, 128).T)

    encx = emb[text.T]                                      # [16, 4096, 50]
    dec_ch = np.concatenate([np.zeros((text.shape[0], 1), text.dtype),
                             text[:, :-1]], axis=1)
    decx = emb[dec_ch.T]                                    # [16, 4096, 50]
    encxT = np.ascontiguousarray(encx.transpose(0, 2, 1))   # [16, 50, 4096]
    decxT = np.ascontiguousarray(decx.transpose(0, 2, 1))
    tgt = np.ascontiguousarray(text.T.astype(f))            # [16, 4096]
    iota = np.arange(128, dtype=f).reshape(128, 1)

    in_maps = []
    for c in range(NCORES):
        sl = slice(c * B, (c + 1) * B)
        in_maps.append({
            "visT": np.ascontiguousarray(visT[:, sl]).reshape(VISK, 128, B).astype(bf),
            "WvisT": WvisT.reshape(VISK, 128, H).astype(bf),
            "WihT": WihT.astype(bf),
            "WhhT": WhhT.reshape(HK, 128, G).astype(bf),
            "biasg": biasg,
            "encx": np.ascontiguousarray(encxT[:, :, sl]).astype(bf),
            "decx": np.ascontiguousarray(decxT[:, :, sl]).astype(bf),
            "WencT": WencT.reshape(HK, 128, H).astype(bf),
            "benc": benc,
            "WoutT": WoutT.reshape(HK, 128, V).astype(bf),
            "tgt": np.ascontiguousarray(tgt[:, sl]),
            "iota128": iota,
            "ones128": np.ones((128, 1), np.float32),
        })
    return in_maps


LAST_EXEC_TIME_NS = None


def kernel(**inputs):
    global LAST_EXEC_TIME_NS
    from concourse.bass_utils import run_bass_kernel_spmd

    if "nc" not in _CACHE:
        _CACHE["nc"] = _build()
    nc = _CACHE["nc"]
    in_maps = _prep_inputs(**inputs)

    trace = bool(int(os.environ.get("KERNEL_PROFILE", "0")))
    kw = {}
    if trace:
        _install_profile_hook()
        kw["trace"] = True
    res = run_bass_kernel_spmd(nc, in_maps, core_ids=list(range(NCORES)), **kw)
    LAST_EXEC_TIME_NS = res.exec_time_ns
    out = np.concatenate([res.results[c]["loss"][0] for c in range(NCORES)])
    return out.astype(np.float32)


def _install_profile_hook():
    """Optional NTFF profiling (dev only; used when KERNEL_PROFILE=1)."""
    import types, ctypes, contextlib
    try:
        import antenv
    except ImportError:
        return
    if getattr(antenv, "axon_hooks", None) is not None:
        return
    mod = types.ModuleType('antenv.axon_hooks')
    _store = [None]
    mod.set_axon_ntff_profile_hook = lambda h: _store.__setitem__(0, h)
    mod.get_axon_ntff_profile_hook = lambda: _store[0]
    sys.modules['antenv.axon_hooks'] = mod
    antenv.axon_hooks = mod
    try:
        lib = ctypes.CDLL('/opt/axon/libaxon_pjrt.so')
    except OSError:
        return
    if not hasattr(lib, 'axon_start_nrt_profile'):
        return
    lib.axon_start_nrt_profile.argtypes = [ctypes.POINTER(ctypes.c_int64),
                                           ctypes.c_size_t]
    lib.axon_start_nrt_profile.restype = ctypes.c_int64
    lib.axon_stop_nrt_profile.argtypes = [ctypes.c_char_p]
    lib.axon_stop_nrt_profile.restype = ctypes.c_int64

    @contextlib.contextmanager
    def _hook(output_dir, device_ids):
        import jax
        jax.devices()
        if device_ids:
            ids = (ctypes.c_int64 * len(device_ids))(*device_ids)
            rc = lib.axon_start_nrt_profile(ids, len(device_ids))
        else:
            rc = lib.axon_start_nrt_profile(None, 0)
        if rc != 0:
            raise RuntimeError(f"axon_start_nrt_profile rc={rc}")
        try:
            yield
        finally:
            n = lib.axon_stop_nrt_profile(str(output_dir).encode())
            print(f"profile: {n} ntff file(s) in {output_dir}", file=sys.stderr)

    mod.set_axon_ntff_profile_hook(_hook)
    import concourse.bass_utils as bu
    bu.upload_artifacts = lambda tmpdir: "local://" + str(tmpdir)



# revision 4
# speedup vs baseline: 1.0404x; 1.0404x over previous
"""Trainium2 Bass kernel for nn_AttentionModelCharLevel.

Model (per reference): visual linear -> char-encoder LSTM -> linear+relu ->
cosine attention (softmax over batch dim) -> char-decoder LSTM -> per-sample
mean NLL over L steps.

Sharding: data-parallel over batch B=4096 across 8 cores (512 rows each).
The [B,B] attention needs every core to see all normalized visual rows, so
each core computes its vn shard (+transpose), AllGathers both, and streams
the gathered rows back through SBUF during the attention phase.

Key device-side conventions (v2, fp8 DoubleRow):
- The LSTM recurrence, encoder linear and decoder logits matmuls run in
  fp8e4 with perf_mode=DoubleRow (2 fp8 K-rows per PE cell): K=512
  contractions become 2 matmuls of logical K=256. Weights are pre-scaled
  x64 host-side (fp8 normal range) and the x1/64 is folded into the ACT
  scale that reads the PSUM.
- Hidden state is stored doubled (Ht = 2h) as a single [128, 4, B] fp8
  tile per step; slice [:, 2g:2g+2, :] is the DoubleRow rhs pair. Cell
  state St = 2c is a [128, 4, B] bf16 tile.
- sigmoid(z) = 0.5*tanh(z/2)+0.5; the g-gate's weights carry an extra x2
  so ALL gates share one ACT scale (0.5/64). The bias is folded into the
  x-part matmul as a 51st embedding row (x=1), so one ACT with no bias
  covers a whole 4-bank PSUM quad [128, 4, B] = the 4 gates of one hidden
  chunk -> 4 gate ACTs + 1 tanh(c) ACT per step instead of 20.
- The DVE tail chain per hidden chunk runs in bf16 (2x DVE rate):
      m2 = (tanh_i + 1) * tanh_g
      m1 = (tanh_f + 1) * S
      S' = 0.5*m1 + m2
      H' = (tanh_o + 1) * tanh(0.5*S')   (fp8 out)
- Softmax over the batch dim reduces to exp() only: sims are cosine
  similarities in [-1,1] and the softmax denominator is a positive
  per-column scale that the final row normalization of h cancels.
- Decoder log-softmax: logits kept in [V, B] layout; Z = ones^T exp(logits)
  and the target logit via a one-hot mask built from an iota column --
  both reduce over partitions with K=128 ones-matmuls into spare banks of
  the rotating PSUM quads. ln() deferred to a single post-pass.
"""
import os
import sys

sys.path.insert(0, '/opt/trn_rl_repo')

import numpy as np

B_FULL = 4096
NCORES = 8
B = B_FULL // NCORES          # 512 rows per core
H = 512
G = 4 * H                     # 2048
E = 50
EA = E + 1                    # embedding dim + bias row
L = 16
V = 128
VIS = 2048
HK = H // 128                 # 4 chunks of the hidden dim
GK = G // 128                 # 16 gate chunks
VISK = VIS // 128             # 16 chunks of the visual dim
BK = B // 128                 # 4 batch chunks per core
VCHUNKS = B_FULL // 128       # 32 chunks of the full batch
SF = 64.0                     # fp8 weight scale

_CACHE = {}


def _build():
    import concourse.bass as bass
    import concourse.tile as tile
    import concourse.mybir as mybir
    from concourse import bacc
    from concourse.masks import make_identity
    from contextlib import ExitStack

    dt = mybir.dt
    AF = mybir.ActivationFunctionType
    ALU = mybir.AluOpType
    DR = mybir.MatmulPerfMode.DoubleRow
    f32 = dt.float32
    f32r = dt.float32r
    bf16d = dt.bfloat16
    f8 = dt.float8e4
    LN2 = float(np.log(2.0))

    AP = bass.AP
    nc = bacc.Bacc("TRN2", target_bir_lowering=False, debug=False,
                   num_devices=NCORES)

    # ---- DRAM I/O ----
    visT_d = nc.dram_tensor("visT", [VISK, 128, B], bf16d, kind="ExternalInput").ap()
    WvisT_d = nc.dram_tensor("WvisT", [VISK, 128, H], bf16d, kind="ExternalInput").ap()
    WihT_d = nc.dram_tensor("WihT", [EA, G], bf16d, kind="ExternalInput").ap()
    Whh8_d = nc.dram_tensor("Whh8", [128, HK, G], f8, kind="ExternalInput").ap()
    encx_d = nc.dram_tensor("encx", [L, EA, B], bf16d, kind="ExternalInput").ap()
    decx_d = nc.dram_tensor("decx", [L, EA, B], bf16d, kind="ExternalInput").ap()
    Wenc8_d = nc.dram_tensor("Wenc8", [128, HK, H], f8, kind="ExternalInput").ap()
    benc_d = nc.dram_tensor("benc", [128, HK], f32, kind="ExternalInput").ap()
    Wout8_d = nc.dram_tensor("Wout8", [128, HK, V], f8, kind="ExternalInput").ap()
    tgt_d = nc.dram_tensor("tgt", [L, B], f32, kind="ExternalInput").ap()
    iota_d = nc.dram_tensor("iota128", [128, 1], f32, kind="ExternalInput").ap()
    ones_d = nc.dram_tensor("ones128", [128, 1], f32r, kind="ExternalInput").ap()
    out_d = nc.dram_tensor("loss", [1, B], f32, kind="ExternalOutput").ap()

    with tile.TileContext(nc) as tc, ExitStack() as top:
        wpool = top.enter_context(tc.tile_pool(name="w", bufs=1))
        spool = top.enter_context(tc.tile_pool(name="state", bufs=2))
        dram = top.enter_context(tc.tile_pool(name="dram", bufs=1, space="DRAM"))

        # ---- persistent weights / constants ----
        Whh8 = wpool.tile([128, HK, G], f8, tag="whh8", name="whh8")
        nc.sync.dma_start(Whh8[:], Whh8_d)
        WihT2 = wpool.tile([64 + EA, G], bf16d, tag="wih", name="wih")
        nc.sync.dma_start(WihT2[:EA, :], WihT_d)
        nc.sync.dma_start(WihT2[64:64 + EA, :], WihT_d)
        Wenc8 = wpool.tile([128, HK, H], f8, tag="wenc8", name="wenc8")
        nc.sync.dma_start(Wenc8[:], Wenc8_d)
        benc = wpool.tile([128, HK], f32, tag="benc", name="benc")
        nc.sync.dma_start(benc[:], benc_d)
        Wout8 = wpool.tile([128, HK, V], f8, tag="wout8", name="wout8")
        nc.sync.dma_start(Wout8[:], Wout8_d)
        iota_c = wpool.tile([128, 1], f32, tag="iota", name="iota")
        nc.sync.dma_start(iota_c[:], iota_d)
        ones_col = wpool.tile([128, 1], f32r, tag="ones_col", name="ones_col")
        nc.sync.dma_start(ones_col[:], ones_d)
        ones_row = wpool.tile([1, 128], f32r, tag="ones_row", name="ones_row")
        nc.sync.dma_start(ones_row[:], ones_d.rearrange("p one -> one p"))
        ones16 = wpool.tile([L, 1], f32r, tag="ones16", name="ones16")
        nc.sync.dma_start(ones16[:], ones_d[:L])
        ident = wpool.tile([128, 128], bf16d, tag="ident", name="ident")
        make_identity(nc, ident[:])
        ln2_t = wpool.tile([1, 1], f32, tag="ln2", name="ln2")
        nc.vector.memset(ln2_t[:], LN2)

        # AllGather buffers: vn blocks and vnT blocks (both bf16)
        ag_in = dram.tile([BK, 128, B], bf16d, name="ag_in")
        ag_out = dram.tile([NCORES, BK, 128, B], bf16d, addr_space="Shared", name="ag_out")
        agt_in = dram.tile([HK, 128, B], bf16d, name="agt_in")
        agt_out = dram.tile([NCORES, HK, 128, B], bf16d, addr_space="Shared", name="agt_out")

        # decoder per-step Z and target-logit rows
        zpool = top.enter_context(tc.tile_pool(name="zp", bufs=1))
        Zs = zpool.tile([L, B], f32, tag="Zs", name="Zs")
        lts = zpool.tile([L, B], f32r, tag="lts", name="lts")

        # ======== Phase 1: visual linear + row-normalize + transpose + AG ====
        with ExitStack() as ph:
            vsb = ph.enter_context(tc.tile_pool(name="vsb", bufs=3))
            vps = ph.enter_context(tc.tile_pool(name="vps", bufs=1, space="PSUM"))
            tps = ph.enter_context(tc.tile_pool(name="tps", bufs=2, space="PSUM"))
            vnpool = ph.enter_context(tc.tile_pool(name="vnp", bufs=1))

            v_ps = [vps.tile([128, H], f32, tag=f"vps{b}", name=f"vps{b}") for b in range(BK)]
            for ki in range(VISK):
                vis_t = vsb.tile([128, B], bf16d, tag="vis", name="vis")
                nc.sync.dma_start(vis_t[:], visT_d[ki])
                wv_t = vsb.tile([128, H], bf16d, tag="wvis", name="wvis")
                nc.sync.dma_start(wv_t[:], WvisT_d[ki])
                for b in range(BK):
                    nc.tensor.matmul(v_ps[b][:], vis_t[:, b * 128:(b + 1) * 128],
                                     wv_t[:], start=(ki == 0), stop=(ki == VISK - 1))
            vn = []
            for b in range(BK):
                sq = vsb.tile([128, H], f32, tag="vsq", name="vsq")
                s_col = vsb.tile([128, 1], f32, tag="vscol", name="vscol")
                nc.scalar.activation(sq[:], v_ps[b][:], AF.Square,
                                     accum_out=s_col[:])
                lnc_ = vsb.tile([128, 1], f32, tag="vln", name="vln")
                nc.scalar.activation(lnc_[:], s_col[:], AF.Ln)
                rs = vsb.tile([128, 1], f32, tag="vrs", name="vrs")
                nc.scalar.activation(rs[:], lnc_[:], AF.Exp, scale=-0.5)
                vn_b = vnpool.tile([128, H], bf16d, tag=f"vn{b}", name=f"vn{b}")
                nc.vector.tensor_scalar(vn_b[:], v_ps[b][:], rs[:], None, ALU.mult)
                vn.append(vn_b)
            # transpose vn -> vnT (16 128x128 blocks, bf16 for the sims lhsT)
            vnT = [vnpool.tile([128, B], bf16d, tag=f"vnT{h}", name=f"vnT{h}") for h in range(HK)]
            for b in range(BK):
                for h in range(HK):
                    t_ps = tps.tile([128, 128], bf16d, tag="tr", name="tr")
                    nc.tensor.transpose(
                        t_ps[:], vn[b][:, h * 128:(h + 1) * 128], ident[:])
                    nc.vector.tensor_copy(vnT[h][:, b * 128:(b + 1) * 128], t_ps[:])
            for b in range(BK):
                nc.sync.dma_start(ag_in[b], vn[b][:])
            for h in range(HK):
                nc.sync.dma_start(agt_in[h], vnT[h][:])
            nc.gpsimd.collective_compute(
                "AllGather", mybir.AluOpType.bypass,
                replica_groups=[list(range(NCORES))],
                ins=[ag_in[:]], outs=[ag_out[:]],
            )
            nc.gpsimd.collective_compute(
                "AllGather", mybir.AluOpType.bypass,
                replica_groups=[list(range(NCORES))],
                ins=[agt_in[:]], outs=[agt_out[:]],
            )

        # staged full vnT (bf16) for the attention sims lhsT; filled by
        # gpsimd DMAs that wait on the collective and run during the encoder
        vnT_all = [wpool.tile([128, B_FULL], bf16d, tag=f"vnTall{k}", name=f"vnTall{k}")
                   for k in range(HK)]
        for k in range(HK):
            for r in range(NCORES):
                nc.gpsimd.dma_start(vnT_all[k][:, r * B:(r + 1) * B], agt_out[r, k])

        # ======== LSTM scan helper ========
        xsb = top.enter_context(tc.tile_pool(name="xsb", bufs=3))
        gsb = top.enter_context(tc.tile_pool(name="gsb", bufs=2))
        msb = top.enter_context(tc.tile_pool(name="msb", bufs=2))

        from collections import deque

        def lstm_step(gps, x_src, Hp8, Sp, max_open=2):
            """One LSTM step, fp8 DoubleRow recurrence, quad-bank PSUM.

            Per hidden chunk j one 4-bank PSUM quad holds gates i,f,g,o.
            open = x-part matmuls (K=51, two concurrent row groups) plus the
            h01 DoubleRow pair; close = h23 pair + one mega-ACT over the quad
            + the bf16 DVE state chain. Closes lag opens so the PE streams
            the next chunk while the previous drains through ACT/DVE, and
            the first closed chunks feed the next step's opens.
            """
            xt = xsb.tile([64 + EA, B], bf16d, tag="xt", name="xt")
            nc.sync.dma_start(xt[:EA, :], x_src)
            nc.sync.dma_start(xt[64:64 + EA, :], x_src)
            Hn8 = spool.tile([128, HK, B], f8, tag="H", name="H")
            Sn = spool.tile([128, HK, B], bf16d, tag="S", name="S")

            def open_chunk(j):
                quad = gps.tile([128, 4, B], f32, tag="quad", name="quad")
                for gate in range(4):
                    c = gate * 4 + j
                    r0 = 0 if gate % 2 == 0 else 64
                    nc.tensor.matmul(quad[:, gate, :],
                                     WihT2[r0:r0 + EA, c * 128:(c + 1) * 128],
                                     xt[r0:r0 + EA, :], start=True, stop=False)
                for gate in range(4):
                    c = gate * 4 + j
                    nc.tensor.matmul(quad[:, gate, :],
                                     Whh8[:, 0:2, c * 128:(c + 1) * 128],
                                     Hp8[:, 0:2, :], start=False, stop=False,
                                     perf_mode=DR)
                return j, quad

            def close_chunk(entry):
                j, quad = entry
                for gate in range(4):
                    c = gate * 4 + j
                    nc.tensor.matmul(quad[:, gate, :],
                                     Whh8[:, 2:4, c * 128:(c + 1) * 128],
                                     Hp8[:, 2:4, :], start=False, stop=True,
                                     perf_mode=DR)
                T = gsb.tile([128, 4, B], bf16d, tag="T", name="T")
                nc.scalar.activation(T[:], quad[:], AF.Tanh, scale=0.5 / SF)
                m2 = msb.tile([128, B], bf16d, tag="m2", name="m2")
                nc.vector.scalar_tensor_tensor(m2[:], T[:, 0, :], 1.0,
                                               T[:, 2, :], ALU.add, ALU.mult)
                m1 = msb.tile([128, B], bf16d, tag="m1", name="m1")
                nc.vector.scalar_tensor_tensor(m1[:], T[:, 1, :], 1.0,
                                               Sp[:, j, :], ALU.add, ALU.mult)
                nc.vector.scalar_tensor_tensor(Sn[:, j, :], m1[:], 0.5, m2[:],
                                               ALU.mult, ALU.add)
                th = msb.tile([128, B], bf16d, tag="th", name="th")
                nc.scalar.activation(th[:], Sn[:, j, :], AF.Tanh, scale=0.5)
                nc.vector.scalar_tensor_tensor(Hn8[:, j, :], T[:, 3, :], 1.0,
                                               th[:], ALU.add, ALU.mult)

            open_q = deque()
            for j in range(HK):
                if len(open_q) >= max_open:
                    close_chunk(open_q.popleft())
                open_q.append(open_chunk(j))
            while open_q:
                close_chunk(open_q.popleft())
            return Hn8, Sn

        # ======== Phase 2: encoder ========
        Hp8 = spool.tile([128, HK, B], f8, tag="H", name="H")
        Sp = spool.tile([128, HK, B], bf16d, tag="S", name="S")
        nc.vector.memset(Hp8[:], 0.2)
        nc.vector.memset(Sp[:], 0.2)
        with tc.tile_pool(name="gpse", bufs=2, space="PSUM") as gps_e:
            for s in range(L):
                Hp8, Sp = lstm_step(gps_e, encx_d[s], Hp8, Sp)
        Henc = Hp8

        # ======== Phase 3: t path + attention ========
        with ExitStack() as ph:
            asb = ph.enter_context(tc.tile_pool(name="asb", bufs=2))
            aps = ph.enter_context(tc.tile_pool(name="aps", bufs=1, space="PSUM"))
            sps_pool = ph.enter_context(tc.tile_pool(name="sps", bufs=2, space="PSUM"))
            vstr = ph.enter_context(tc.tile_pool(name="vstr", bufs=3))

            # t = relu(Wenc' @ Henc + benc), column-normalized
            tr = []
            s_ps = aps.tile([1, B], f32, tag="tsum", name="tsum")
            for mi in range(HK):
                t_ps = sps_pool.tile([128, B], f32, tag="sims", name="sims")
                nc.tensor.matmul(t_ps[:], Wenc8[:, 0:2, mi * 128:(mi + 1) * 128],
                                 Henc[:, 0:2, :], start=True, stop=False,
                                 perf_mode=DR)
                nc.tensor.matmul(t_ps[:], Wenc8[:, 2:4, mi * 128:(mi + 1) * 128],
                                 Henc[:, 2:4, :], start=False, stop=True,
                                 perf_mode=DR)
                tr_mi = asb.tile([128, B], f32, tag=f"tr{mi}", name=f"tr{mi}", bufs=1)
                nc.scalar.activation(tr_mi[:], t_ps[:], AF.Relu, scale=1.0 / SF,
                                     bias=benc[:, mi:mi + 1])
                tr.append(tr_mi)
                sq = asb.tile([128, B], f32r, tag="tsq", name="tsq")
                nc.scalar.activation(sq[:], tr_mi[:], AF.Square)
                nc.tensor.matmul(s_ps[:], ones_col[:], sq[:],
                                 start=(mi == 0), stop=(mi == HK - 1))
            lnr = asb.tile([1, B], f32, tag="tlnr", name="tlnr")
            nc.scalar.activation(lnr[:], s_ps[:], AF.Ln)
            rs_r = asb.tile([1, B], f32r, tag="trs", name="trs")
            nc.scalar.activation(rs_r[:], lnr[:], AF.Exp, scale=-0.5)
            bc_ps = aps.tile([128, B], f32, tag="tbc", name="tbc")
            nc.tensor.matmul(bc_ps[:], ones_row[:], rs_r[:], start=True, stop=True)
            tnT = []
            for mi in range(HK):
                tn_mi = asb.tile([128, B], bf16d, tag=f"tn{mi}", name=f"tn{mi}", bufs=1)
                nc.vector.tensor_tensor(tn_mi[:], tr[mi][:], bc_ps[:], ALU.mult)
                tnT.append(tn_mi)

            # attention: stream the gathered vn/vnT, E=exp(sims), accumulate h
            hu_ps = [aps.tile([128, B], f32, tag=f"hu{h}", name=f"hu{h}") for h in range(HK)]
            for i in range(VCHUNKS):
                r, b = divmod(i, BK)
                vn_i = vstr.tile([128, B], bf16d, tag="vni", name="vni", bufs=4)
                nc.sync.dma_start(vn_i[:], ag_out[r, b])
                sim_ps = sps_pool.tile([128, B], f32, tag="sims", name="sims")
                for k in range(HK):
                    nc.tensor.matmul(sim_ps[:],
                                     vnT_all[k][:, i * 128:(i + 1) * 128],
                                     tnT[k][:],
                                     start=(k == 0), stop=(k == HK - 1))
                E_i = vstr.tile([128, B], bf16d, tag="E", name="E")
                nc.scalar.activation(E_i[:], sim_ps[:], AF.Exp)
                for h in range(HK):
                    nc.tensor.matmul(hu_ps[h][:],
                                     vn_i[:, h * 128:(h + 1) * 128], E_i[:],
                                     start=(i == 0), stop=(i == VCHUNKS - 1))
            # normalize h (x2 for the doubled-state convention) -> decoder init
            s2_ps = aps.tile([1, B], f32, tag="tsum", name="tsum")
            for h in range(HK):
                sq = asb.tile([128, B], f32r, tag="husq", name="husq")
                nc.scalar.activation(sq[:], hu_ps[h][:], AF.Square)
                nc.tensor.matmul(s2_ps[:], ones_col[:], sq[:],
                                 start=(h == 0), stop=(h == HK - 1))
            lnr2 = asb.tile([1, B], f32, tag="hulnr", name="hulnr")
            nc.scalar.activation(lnr2[:], s2_ps[:], AF.Ln)
            rs2 = asb.tile([1, B], f32r, tag="hurs", name="hurs")
            nc.scalar.activation(rs2[:], lnr2[:], AF.Exp, scale=-0.5, bias=ln2_t[:])
            bc2_ps = aps.tile([128, B], f32, tag="tbc", name="tbc")
            nc.tensor.matmul(bc2_ps[:], ones_row[:], rs2[:], start=True, stop=True)
            bc2_sb = asb.tile([128, B], f32, tag="bc2sb", name="bc2sb", bufs=1)
            nc.vector.tensor_copy(bc2_sb[:], bc2_ps[:])
            H0 = spool.tile([128, HK, B], f8, tag="H", name="H")
            S0 = spool.tile([128, HK, B], bf16d, tag="S", name="S")
            for h in range(HK):
                nc.vector.tensor_tensor(H0[:, h, :], hu_ps[h][:], bc2_sb[:], ALU.mult)
                nc.vector.tensor_tensor(S0[:, h, :], hu_ps[h][:], bc2_sb[:], ALU.mult)

        # ======== Phase 4: decoder ========
        dsb = top.enter_context(tc.tile_pool(name="dsb", bufs=2))
        with tc.tile_pool(name="gpsd", bufs=2, space="PSUM") as gps_d:
            Hp8, Sp = H0, S0
            for s in range(L):
                Hp8, Sp = lstm_step(gps_d, decx_d[s], Hp8, Sp)
                # logitsT [V, B] in slice 0 of a rotating quad; Z and target
                # logit reductions land in spare banks 1 and 2
                lq = gps_d.tile([128, 4, B], f32, tag="quad", name="quad")
                nc.tensor.matmul(lq[:, 0, :], Wout8[:, 0:2, :], Hp8[:, 0:2, :],
                                 start=True, stop=False, perf_mode=DR)
                nc.tensor.matmul(lq[:, 0, :], Wout8[:, 2:4, :], Hp8[:, 2:4, :],
                                 start=False, stop=True, perf_mode=DR)
                El = dsb.tile([128, B], f32r, tag="El", name="El")
                nc.scalar.activation(El[:], lq[:, 0, :], AF.Exp, scale=1.0 / SF)
                nc.tensor.matmul(lq[0:1, 1, :], ones_col[:], El[:],
                                 start=True, stop=True)
                ztmp = dsb.tile([1, B], f32, tag="ztmp", name="ztmp")
                nc.vector.tensor_copy(ztmp[:], lq[0:1, 1, :])
                nc.sync.dma_start(Zs[s:s + 1, :], ztmp[:])
                # target logit via iota==tgt mask (x SF, undone in the post-pass)
                tb = dsb.tile([128, B], f32, tag="tb", name="tb")
                tb_src = AP(tensor=tgt_d.tensor, offset=s * B,
                            ap=[[0, 128], [1, B]])
                nc.gpsimd.dma_start(tb[:], tb_src)
                mk = dsb.tile([128, B], f32r, tag="mk", name="mk")
                nc.vector.scalar_tensor_tensor(mk[:], tb[:], iota_c[:],
                                               lq[:, 0, :], ALU.is_equal, ALU.mult)
                nc.tensor.matmul(lq[0:1, 2, :], ones_col[:], mk[:],
                                 start=True, stop=True)
                lttmp = dsb.tile([1, B], f32r, tag="lttmp", name="lttmp")
                nc.vector.tensor_copy(lttmp[:], lq[0:1, 2, :])
                nc.sync.dma_start(lts[s:s + 1, :], lttmp[:])

            # ======== Phase 5: final loss ========
            lnZ = dsb.tile([L, B], f32r, tag="lnZ", name="lnZ")
            nc.scalar.activation(lnZ[:], Zs[:], AF.Ln)
            diff = dsb.tile([L, B], f32r, tag="diff", name="diff")
            nc.vector.scalar_tensor_tensor(diff[:], lts[:], 1.0 / SF, lnZ[:],
                                           ALU.mult, ALU.subtract)
            fq = gps_d.tile([128, 4, B], f32, tag="quad", name="quad")
            nc.tensor.matmul(fq[0:1, 0, :], ones16[:], diff[:], start=True, stop=True)
            loss_sb = dsb.tile([1, B], f32, tag="losssb", name="losssb")
            nc.vector.tensor_scalar(loss_sb[:], fq[0:1, 0, :], -1.0 / L, None,
                                    ALU.mult)
            nc.sync.dma_start(out_d, loss_sb[:])

    nc.compile()
    return nc


def _prep_inputs(visual_input, text_input, emb, W_ih, W_hh, b_ih, b_hh,
                 W_enc, b_enc, W_out, W_vis):
    import ml_dtypes
    bf = ml_dtypes.bfloat16
    f8n = ml_dtypes.float8_e4m3
    f = np.float32
    vis = np.asarray(visual_input, f)[:, 0, :]              # [4096, 2048]
    text = np.asarray(text_input)
    emb = np.asarray(emb, f)
    visT = np.ascontiguousarray(vis.T)                      # [2048, 4096]
    WvisT = np.ascontiguousarray(np.asarray(W_vis, f).T)    # [2048, 512]

    # x-part weights x64 with bias folded as row 50; g-gate block x2 so all
    # gates share the ACT scale 0.5/64
    b = np.asarray(b_ih, f) + np.asarray(b_hh, f)           # [2048]
    WihT = np.concatenate([np.asarray(W_ih, f).T, b[None, :]], axis=0) * SF
    WihT[:, 2 * H:3 * H] *= 2.0                             # [51, 2048]

    def pack8(WT):                                          # [512, M] -> [128, 4, M]
        M = WT.shape[1]
        return np.ascontiguousarray(
            WT.reshape(HK, 128, M).transpose(1, 0, 2)).astype(f8n)

    WhhT = (0.5 * SF) * np.asarray(W_hh, f).T               # [512, 2048]
    WhhT[:, 2 * H:3 * H] *= 2.0
    Whh8 = pack8(WhhT)
    Wenc8 = pack8((0.5 * SF) * np.asarray(W_enc, f).T)      # [512, 512]
    Wout8 = pack8((0.5 * SF) * np.asarray(W_out, f).T)      # [512, 128]
    benc = np.ascontiguousarray(np.asarray(b_enc, f).reshape(HK, 128).T)

    encx = emb[text.T]                                      # [16, 4096, 50]
    dec_ch = np.concatenate([np.zeros((text.shape[0], 1), text.dtype),
                             text[:, :-1]], axis=1)
    decx = emb[dec_ch.T]                                    # [16, 4096, 50]
    one_row = np.ones((L, 1, B_FULL), f)
    encxT = np.concatenate([encx.transpose(0, 2, 1), one_row], axis=1)
    decxT = np.concatenate([decx.transpose(0, 2, 1), one_row], axis=1)
    encxT = np.ascontiguousarray(encxT)                     # [16, 51, 4096]
    decxT = np.ascontiguousarray(decxT)
    tgt = np.ascontiguousarray(text.T.astype(f))            # [16, 4096]
    iota = np.arange(128, dtype=f).reshape(128, 1)

    in_maps = []
    for c in range(NCORES):
        sl = slice(c * B, (c + 1) * B)
        in_maps.append({
            "visT": np.ascontiguousarray(visT[:, sl]).reshape(VISK, 128, B).astype(bf),
            "WvisT": WvisT.reshape(VISK, 128, H).astype(bf),
            "WihT": WihT.astype(bf),
            "Whh8": Whh8,
            "encx": np.ascontiguousarray(encxT[:, :, sl]).astype(bf),
            "decx": np.ascontiguousarray(decxT[:, :, sl]).astype(bf),
            "Wenc8": Wenc8,
            "benc": benc,
            "Wout8": Wout8,
            "tgt": np.ascontiguousarray(tgt[:, sl]),
            "iota128": iota,
            "ones128": np.ones((128, 1), np.float32),
        })
    return in_maps


LAST_EXEC_TIME_NS = None


def kernel(**inputs):
    global LAST_EXEC_TIME_NS
    from concourse.bass_utils import run_bass_kernel_spmd

    if "nc" not in _CACHE:
        _CACHE["nc"] = _build()
    nc = _CACHE["nc"]
    in_maps = _prep_inputs(**inputs)

    trace = bool(int(os.environ.get("KERNEL_PROFILE", "0")))
    kw = {}
    if trace:
        _install_profile_hook()
        kw["trace"] = True
    res = run_bass_kernel_spmd(nc, in_maps, core_ids=list(range(NCORES)), **kw)
    LAST_EXEC_TIME_NS = res.exec_time_ns
    out = np.concatenate([res.results[c]["loss"][0] for c in range(NCORES)])
    return out.astype(np.float32)


def _install_profile_hook():
    """Optional NTFF profiling (dev only; used when KERNEL_PROFILE=1)."""
    import types, ctypes, contextlib
    try:
        import antenv
    except ImportError:
        return
    if getattr(antenv, "axon_hooks", None) is not None:
        return
    mod = types.ModuleType('antenv.axon_hooks')
    _store = [None]
    mod.set_axon_ntff_profile_hook = lambda h: _store.__setitem__(0, h)
    mod.get_axon_ntff_profile_hook = lambda: _store[0]
    sys.modules['antenv.axon_hooks'] = mod
    antenv.axon_hooks = mod
    try:
        lib = ctypes.CDLL('/opt/axon/libaxon_pjrt.so')
    except OSError:
        return
    if not hasattr(lib, 'axon_start_nrt_profile'):
        return
    lib.axon_start_nrt_profile.argtypes = [ctypes.POINTER(ctypes.c_int64),
                                           ctypes.c_size_t]
    lib.axon_start_nrt_profile.restype = ctypes.c_int64
    lib.axon_stop_nrt_profile.argtypes = [ctypes.c_char_p]
    lib.axon_stop_nrt_profile.restype = ctypes.c_int64

    @contextlib.contextmanager
    def _hook(output_dir, device_ids):
        import jax
        jax.devices()
        if device_ids:
            ids = (ctypes.c_int64 * len(device_ids))(*device_ids)
            rc = lib.axon_start_nrt_profile(ids, len(device_ids))
        else:
            rc = lib.axon_start_nrt_profile(None, 0)
        if rc != 0:
            raise RuntimeError(f"axon_start_nrt_profile rc={rc}")
        try:
            yield
        finally:
            n = lib.axon_stop_nrt_profile(str(output_dir).encode())
            print(f"profile: {n} ntff file(s) in {output_dir}", file=sys.stderr)

    mod.set_axon_ntff_profile_hook = mod.set_axon_ntff_profile_hook
    mod.set_axon_ntff_profile_hook(_hook)
    import concourse.bass_utils as bu
    bu.upload_artifacts = lambda tmpdir: "local://" + str(tmpdir)


# revision 21
# speedup vs baseline: 1.1294x; 1.0855x over previous
"""Trainium2 Bass kernel for nn_AttentionModelCharLevel.

Model (per reference): visual linear -> char-encoder LSTM -> linear+relu ->
cosine attention (softmax over batch dim) -> char-decoder LSTM -> per-sample
mean NLL over L steps.

Sharding: data-parallel over batch B=4096 across 8 cores (512 rows each).
The [B,B] attention needs every core to see all normalized visual rows, so
each core computes its vn shard (+transpose), AllGathers both, and streams
the gathered rows back through SBUF during the attention phase.

Key device-side conventions (v2, fp8 DoubleRow):
- The LSTM recurrence, encoder linear and decoder logits matmuls run in
  fp8e4 with perf_mode=DoubleRow (2 fp8 K-rows per PE cell): K=512
  contractions become 2 matmuls of logical K=256. Weights are pre-scaled
  x64 host-side (fp8 normal range) and the x1/64 is folded into the ACT
  scale that reads the PSUM.
- Hidden state is stored doubled (Ht = 2h) as a single [128, 4, B] fp8
  tile per step; slice [:, 2g:2g+2, :] is the DoubleRow rhs pair. Cell
  state St = 2c is a [128, 4, B] bf16 tile.
- sigmoid(z) = 0.5*tanh(z/2)+0.5; the g-gate's weights carry an extra x2
  so ALL gates share one ACT scale (0.5/64). The bias is folded into the
  x-part matmul as a 51st embedding row (x=1), so one ACT with no bias
  covers a whole 4-bank PSUM quad [128, 4, B] = the 4 gates of one hidden
  chunk -> 4 gate ACTs + 1 tanh(c) ACT per step instead of 20.
- The DVE tail chain per hidden chunk runs in bf16 (2x DVE rate):
      m2 = (tanh_i + 1) * tanh_g
      m1 = (tanh_f + 1) * S
      S' = 0.5*m1 + m2
      H' = (tanh_o + 1) * tanh(0.5*S')   (fp8 out)
- Softmax over the batch dim reduces to exp() only: sims are cosine
  similarities in [-1,1] and the softmax denominator is a positive
  per-column scale that the final row normalization of h cancels.
- Decoder log-softmax: logits kept in [V, B] layout; Z = ones^T exp(logits)
  and the target logit via a one-hot mask built from an iota column --
  both reduce over partitions with K=128 ones-matmuls into spare banks of
  the rotating PSUM quads. ln() deferred to a single post-pass.
"""
import os
import sys

sys.path.insert(0, '/opt/trn_rl_repo')

import numpy as np

B_FULL = 4096
NCORES = 8
B = B_FULL // NCORES          # 512 rows per core
H = 512
G = 4 * H                     # 2048
E = 50
EA = E + 1                    # embedding dim + bias row
L = 16
V = 128
VIS = 2048
HK = H // 128                 # 4 chunks of the hidden dim
GK = G // 128                 # 16 gate chunks
VISK = VIS // 128             # 16 chunks of the visual dim
BK = B // 128                 # 4 batch chunks per core
VCHUNKS = B_FULL // 128       # 32 chunks of the full batch
SF = 64.0                     # fp8 weight scale

_CACHE = {}


def _build():
    import concourse.bass as bass
    import concourse.tile as tile
    import concourse.mybir as mybir
    from concourse import bacc
    from concourse.masks import make_identity
    from contextlib import ExitStack

    dt = mybir.dt
    AF = mybir.ActivationFunctionType
    ALU = mybir.AluOpType
    DR = mybir.MatmulPerfMode.DoubleRow
    f32 = dt.float32
    f32r = dt.float32r
    bf16d = dt.bfloat16
    f8 = dt.float8e4
    LN2 = float(np.log(2.0))

    AP = bass.AP
    nc = bacc.Bacc("TRN2", target_bir_lowering=False, debug=False,
                   num_devices=NCORES)

    # ---- DRAM I/O ----
    visT_d = nc.dram_tensor("visT", [VISK, 128, B], bf16d, kind="ExternalInput").ap()
    WvisT_d = nc.dram_tensor("WvisT", [VISK, 128, H], bf16d, kind="ExternalInput").ap()
    WihT_d = nc.dram_tensor("WihT", [EA, G], bf16d, kind="ExternalInput").ap()
    Whh8_d = nc.dram_tensor("Whh8", [128, HK, G], f8, kind="ExternalInput").ap()
    encx_d = nc.dram_tensor("encx", [L, EA, B], bf16d, kind="ExternalInput").ap()
    decx_d = nc.dram_tensor("decx", [L, EA, B], bf16d, kind="ExternalInput").ap()
    Wenc8_d = nc.dram_tensor("Wenc8", [128, HK, H], f8, kind="ExternalInput").ap()
    benc_d = nc.dram_tensor("benc", [128, HK], f32, kind="ExternalInput").ap()
    Wout8_d = nc.dram_tensor("Wout8", [128, HK, V], f8, kind="ExternalInput").ap()
    tgt_d = nc.dram_tensor("tgt", [L, B], f32, kind="ExternalInput").ap()
    iota_d = nc.dram_tensor("iota128", [128, 1], f32, kind="ExternalInput").ap()
    oneh_d = nc.dram_tensor("oneh", [L, 128, B], f32r, kind="ExternalInput").ap()
    ones_d = nc.dram_tensor("ones128", [128, 1], f32r, kind="ExternalInput").ap()
    out_d = nc.dram_tensor("loss", [1, B], f32, kind="ExternalOutput").ap()

    with tile.TileContext(nc) as tc, ExitStack() as top:
        wpool = top.enter_context(tc.tile_pool(name="w", bufs=1))
        spool = top.enter_context(tc.tile_pool(name="state", bufs=2))
        dram = top.enter_context(tc.tile_pool(name="dram", bufs=1, space="DRAM"))

        # ---- persistent weights / constants ----
        Whh8 = wpool.tile([128, HK, G], f8, tag="whh8", name="whh8")
        nc.sync.dma_start(Whh8[:], Whh8_d)
        WihT2 = wpool.tile([64 + EA, G], bf16d, tag="wih", name="wih")
        nc.sync.dma_start(WihT2[:EA, :], WihT_d)
        nc.sync.dma_start(WihT2[64:64 + EA, :], WihT_d)
        Wenc8 = wpool.tile([128, HK, H], f8, tag="wenc8", name="wenc8")
        nc.sync.dma_start(Wenc8[:], Wenc8_d)
        benc = wpool.tile([128, HK], f32, tag="benc", name="benc")
        nc.sync.dma_start(benc[:], benc_d)
        Wout8 = wpool.tile([128, HK, V], f8, tag="wout8", name="wout8")
        nc.sync.dma_start(Wout8[:], Wout8_d)
        iota_c = wpool.tile([128, 1], f32, tag="iota", name="iota")
        nc.sync.dma_start(iota_c[:], iota_d)

        ones_col = wpool.tile([128, 1], f32r, tag="ones_col", name="ones_col")
        nc.sync.dma_start(ones_col[:], ones_d)
        ones_row = wpool.tile([1, 128], f32r, tag="ones_row", name="ones_row")
        nc.sync.dma_start(ones_row[:], ones_d.rearrange("p one -> one p"))
        ones16 = wpool.tile([L, 1], f32r, tag="ones16", name="ones16")
        nc.sync.dma_start(ones16[:], ones_d[:L])
        ident = wpool.tile([128, 128], bf16d, tag="ident", name="ident")
        make_identity(nc, ident[:])
        ln2_t = wpool.tile([1, 1], f32, tag="ln2", name="ln2")
        nc.vector.memset(ln2_t[:], LN2)

        # prefetch ALL scan inputs up front on the gpsimd DMA queue so the
        # LSTM scans never wait on the sync queue / collective window
        xenc = wpool.tile([64 + EA, L, B], bf16d, tag="xenc", name="xenc")
        xdec = wpool.tile([64 + EA, L, B], bf16d, tag="xdec", name="xdec")
        for s in range(L):
            nc.gpsimd.dma_start(xenc[:EA, s, :], encx_d[s])
            nc.gpsimd.dma_start(xenc[64:64 + EA, s, :], encx_d[s])
        for s in range(L):
            nc.gpsimd.dma_start(xdec[:EA, s, :], decx_d[s])
            nc.gpsimd.dma_start(xdec[64:64 + EA, s, :], decx_d[s])

        # AllGather buffers: vn blocks and vnT blocks (both bf16)
        ag_in = dram.tile([BK, 128, B], bf16d, name="ag_in")
        ag_out = dram.tile([NCORES, BK, 128, B], bf16d, addr_space="Shared", name="ag_out")
        agt_in = dram.tile([HK, 128, B], bf16d, name="agt_in")
        agt_out = dram.tile([NCORES, HK, 128, B], bf16d, addr_space="Shared", name="agt_out")

        # decoder per-step Z and target-logit rows
        zpool = top.enter_context(tc.tile_pool(name="zp", bufs=1))
        Zs = zpool.tile([L, B], f32, tag="Zs", name="Zs")
        lts = zpool.tile([L, B], f32r, tag="lts", name="lts")

        # ======== Phase 1: visual linear + row-normalize + transpose + AG ====
        with ExitStack() as ph:
            vsb = ph.enter_context(tc.tile_pool(name="vsb", bufs=3))
            vps = ph.enter_context(tc.tile_pool(name="vps", bufs=1, space="PSUM"))
            tps = ph.enter_context(tc.tile_pool(name="tps", bufs=2, space="PSUM"))
            vnpool = ph.enter_context(tc.tile_pool(name="vnp", bufs=1))

            v_ps = [vps.tile([128, H], f32, tag=f"vps{b}", name=f"vps{b}") for b in range(BK)]
            for ki in range(VISK):
                vis_t = vsb.tile([128, B], bf16d, tag="vis", name="vis")
                nc.sync.dma_start(vis_t[:], visT_d[ki])
                wv_t = vsb.tile([128, H], bf16d, tag="wvis", name="wvis")
                nc.sync.dma_start(wv_t[:], WvisT_d[ki])
                for b in range(BK):
                    nc.tensor.matmul(v_ps[b][:], vis_t[:, b * 128:(b + 1) * 128],
                                     wv_t[:], start=(ki == 0), stop=(ki == VISK - 1))
            vn = []
            for b in range(BK):
                sq = vsb.tile([128, H], f32, tag="vsq", name="vsq")
                s_col = vsb.tile([128, 1], f32, tag="vscol", name="vscol")
                nc.scalar.activation(sq[:], v_ps[b][:], AF.Square,
                                     accum_out=s_col[:])
                lnc_ = vsb.tile([128, 1], f32, tag="vln", name="vln")
                nc.scalar.activation(lnc_[:], s_col[:], AF.Ln)
                rs = vsb.tile([128, 1], f32, tag="vrs", name="vrs")
                nc.scalar.activation(rs[:], lnc_[:], AF.Exp, scale=-0.5)
                vn_b = vnpool.tile([128, H], bf16d, tag=f"vn{b}", name=f"vn{b}")
                nc.vector.tensor_scalar(vn_b[:], v_ps[b][:], rs[:], None, ALU.mult)
                vn.append(vn_b)
            # transpose vn -> vnT (16 128x128 blocks, bf16 for the sims lhsT)
            vnT = [vnpool.tile([128, B], bf16d, tag=f"vnT{h}", name=f"vnT{h}") for h in range(HK)]
            for b in range(BK):
                for h in range(HK):
                    t_ps = tps.tile([128, 128], bf16d, tag="tr", name="tr")
                    nc.tensor.transpose(
                        t_ps[:], vn[b][:, h * 128:(h + 1) * 128], ident[:])
                    nc.vector.tensor_copy(vnT[h][:, b * 128:(b + 1) * 128], t_ps[:])
            for b in range(BK):
                nc.sync.dma_start(ag_in[b], vn[b][:])
            for h in range(HK):
                nc.sync.dma_start(agt_in[h], vnT[h][:])
            nc.gpsimd.collective_compute(
                "AllGather", mybir.AluOpType.bypass,
                replica_groups=[list(range(NCORES))],
                ins=[ag_in[:]], outs=[ag_out[:]],
            )
            nc.gpsimd.collective_compute(
                "AllGather", mybir.AluOpType.bypass,
                replica_groups=[list(range(NCORES))],
                ins=[agt_in[:]], outs=[agt_out[:]],
            )

        # staged full vnT (bf16) for the attention sims lhsT; filled by
        # gpsimd DMAs that wait on the collective and run during the encoder
        vnT_all = [wpool.tile([128, B_FULL], bf16d, tag=f"vnTall{k}", name=f"vnTall{k}")
                   for k in range(HK)]
        for k in range(HK):
            for r in range(NCORES):
                nc.gpsimd.dma_start(vnT_all[k][:, r * B:(r + 1) * B], agt_out[r, k])

        # ======== LSTM scan helper ========
        gsb = top.enter_context(tc.tile_pool(name="gsb", bufs=3))
        msb = top.enter_context(tc.tile_pool(name="msb", bufs=2))

        from collections import deque

        def lstm_step(gps, xall, s, Hp8, Sp, max_open=2):
            """One LSTM step, fp8 DoubleRow recurrence, quad-bank PSUM.

            Per hidden chunk j one 4-bank PSUM quad holds gates i,f,g,o.
            open = x-part matmuls (K=51, two concurrent row groups) plus the
            h01 DoubleRow pair; close = h23 pair + one mega-ACT over the quad
            + the state chain split across gpsimd (m2, m1) and DVE (S', H'),
            with tanh(c) in chunk pairs on ACT. Closes lag opens so the PE
            streams the next chunk while the previous drains, and the first
            closed chunks feed the next step's opens.
            """
            Hn8 = spool.tile([128, HK, B], f8, tag="H", name="H")
            Sn = spool.tile([128, HK, B], bf16d, tag="S", name="S")
            Ts = {}

            def open_chunk(j):
                quad = gps.tile([128, 4, B], f32, tag="quad", name="quad")
                for gate in range(4):
                    c = gate * 4 + j
                    r0 = 0 if gate % 2 == 0 else 64
                    nc.tensor.matmul(quad[:, gate, :],
                                     WihT2[r0:r0 + EA, c * 128:(c + 1) * 128],
                                     xall[r0:r0 + EA, s, :], start=True, stop=False)
                for gate in range(4):
                    c = gate * 4 + j
                    nc.tensor.matmul(quad[:, gate, :],
                                     Whh8[:, 0:2, c * 128:(c + 1) * 128],
                                     Hp8[:, 0:2, :], start=False, stop=False,
                                     perf_mode=DR)
                return j, quad

            def close_chunk(entry):
                j, quad = entry
                for gate in range(4):
                    c = gate * 4 + j
                    nc.tensor.matmul(quad[:, gate, :],
                                     Whh8[:, 2:4, c * 128:(c + 1) * 128],
                                     Hp8[:, 2:4, :], start=False, stop=True,
                                     perf_mode=DR)
                T = gsb.tile([128, 4, B], bf16d, tag="T", name="T")
                Ts[j] = T
                nc.scalar.activation(T[:], quad[:], AF.Tanh, scale=0.5 / SF)
                m2 = msb.tile([128, B], bf16d, tag="m2", name="m2")
                nc.vector.scalar_tensor_tensor(m2[:], T[:, 0, :], 1.0,
                                               T[:, 2, :], ALU.add, ALU.mult)
                m1 = msb.tile([128, B], bf16d, tag="m1", name="m1")
                nc.vector.scalar_tensor_tensor(m1[:], T[:, 1, :], 1.0,
                                               Sp[:, j, :], ALU.add, ALU.mult)
                nc.vector.scalar_tensor_tensor(Sn[:, j, :], m1[:], 0.5, m2[:],
                                               ALU.mult, ALU.add)
                if j % 2 == 1:  # tanh(c) for the chunk pair, then both H'
                    th = msb.tile([128, 2, B], bf16d, tag="th", name="th")
                    nc.scalar.activation(th[:], Sn[:, j - 1:j + 1, :],
                                         AF.Tanh, scale=0.5)
                    for jj in (j - 1, j):
                        nc.vector.scalar_tensor_tensor(
                            Hn8[:, jj, :], Ts[jj][:, 3, :], 1.0,
                            th[:, jj - (j - 1), :], ALU.add, ALU.mult)

            open_q = deque()
            for j in range(HK):
                if len(open_q) >= max_open:
                    close_chunk(open_q.popleft())
                open_q.append(open_chunk(j))
            while open_q:
                close_chunk(open_q.popleft())
            return Hn8, Sn

        # ======== Phase 2: encoder ========
        Hp8 = spool.tile([128, HK, B], f8, tag="H", name="H")
        Sp = spool.tile([128, HK, B], bf16d, tag="S", name="S")
        nc.vector.memset(Hp8[:], 0.2)
        nc.vector.memset(Sp[:], 0.2)
        with tc.tile_pool(name="gpse", bufs=2, space="PSUM") as gps_e:
            for s in range(L):
                Hp8, Sp = lstm_step(gps_e, xenc, s, Hp8, Sp)
        Henc = Hp8

        # ======== Phase 3: t path + attention ========
        with ExitStack() as ph:
            asb = ph.enter_context(tc.tile_pool(name="asb", bufs=2))
            aps = ph.enter_context(tc.tile_pool(name="aps", bufs=1, space="PSUM"))
            sps_pool = ph.enter_context(tc.tile_pool(name="sps", bufs=2, space="PSUM"))
            vstr = ph.enter_context(tc.tile_pool(name="vstr", bufs=3))

            # t = relu(Wenc' @ Henc + benc), column-normalized
            tr = []
            s_ps = aps.tile([1, B], f32, tag="tsum", name="tsum")
            for mi in range(HK):
                t_ps = sps_pool.tile([128, B], f32, tag="sims", name="sims")
                nc.tensor.matmul(t_ps[:], Wenc8[:, 0:2, mi * 128:(mi + 1) * 128],
                                 Henc[:, 0:2, :], start=True, stop=False,
                                 perf_mode=DR)
                nc.tensor.matmul(t_ps[:], Wenc8[:, 2:4, mi * 128:(mi + 1) * 128],
                                 Henc[:, 2:4, :], start=False, stop=True,
                                 perf_mode=DR)
                tr_mi = asb.tile([128, B], f32, tag=f"tr{mi}", name=f"tr{mi}", bufs=1)
                nc.scalar.activation(tr_mi[:], t_ps[:], AF.Relu, scale=1.0 / SF,
                                     bias=benc[:, mi:mi + 1])
                tr.append(tr_mi)
                sq = asb.tile([128, B], f32r, tag="tsq", name="tsq")
                nc.scalar.activation(sq[:], tr_mi[:], AF.Square)
                nc.tensor.matmul(s_ps[:], ones_col[:], sq[:],
                                 start=(mi == 0), stop=(mi == HK - 1))
            lnr = asb.tile([1, B], f32, tag="tlnr", name="tlnr")
            nc.scalar.activation(lnr[:], s_ps[:], AF.Ln)
            rs_r = asb.tile([1, B], f32r, tag="trs", name="trs")
            nc.scalar.activation(rs_r[:], lnr[:], AF.Exp, scale=-0.5)
            bc_ps = aps.tile([128, B], f32, tag="tbc", name="tbc")
            nc.tensor.matmul(bc_ps[:], ones_row[:], rs_r[:], start=True, stop=True)
            tnT = []
            for mi in range(HK):
                tn_mi = asb.tile([128, B], bf16d, tag=f"tn{mi}", name=f"tn{mi}", bufs=1)
                nc.vector.tensor_tensor(tn_mi[:], tr[mi][:], bc_ps[:], ALU.mult)
                tnT.append(tn_mi)

            # attention: stream the gathered vn/vnT, E=exp(sims), accumulate h
            hu_ps = [aps.tile([128, B], f32, tag=f"hu{h}", name=f"hu{h}") for h in range(HK)]
            for i in range(VCHUNKS):
                r, b = divmod(i, BK)
                vn_i = vstr.tile([128, B], bf16d, tag="vni", name="vni", bufs=4)
                nc.sync.dma_start(vn_i[:], ag_out[r, b])
                sim_ps = sps_pool.tile([128, B], f32, tag="sims", name="sims")
                for k in range(HK):
                    nc.tensor.matmul(sim_ps[:],
                                     vnT_all[k][:, i * 128:(i + 1) * 128],
                                     tnT[k][:],
                                     start=(k == 0), stop=(k == HK - 1))
                E_i = vstr.tile([128, B], bf16d, tag="E", name="E")
                nc.scalar.activation(E_i[:], sim_ps[:], AF.Exp)
                for h in range(HK):
                    nc.tensor.matmul(hu_ps[h][:],
                                     vn_i[:, h * 128:(h + 1) * 128], E_i[:],
                                     start=(i == 0), stop=(i == VCHUNKS - 1))
            # normalize h (x2 for the doubled-state convention) -> decoder init
            s2_ps = aps.tile([1, B], f32, tag="tsum", name="tsum")
            for h in range(HK):
                sq = asb.tile([128, B], f32r, tag="husq", name="husq")
                nc.scalar.activation(sq[:], hu_ps[h][:], AF.Square)
                nc.tensor.matmul(s2_ps[:], ones_col[:], sq[:],
                                 start=(h == 0), stop=(h == HK - 1))
            lnr2 = asb.tile([1, B], f32, tag="hulnr", name="hulnr")
            nc.scalar.activation(lnr2[:], s2_ps[:], AF.Ln)
            rs2 = asb.tile([1, B], f32r, tag="hurs", name="hurs")
            nc.scalar.activation(rs2[:], lnr2[:], AF.Exp, scale=-0.5, bias=ln2_t[:])
            bc2_ps = aps.tile([128, B], f32, tag="tbc", name="tbc")
            nc.tensor.matmul(bc2_ps[:], ones_row[:], rs2[:], start=True, stop=True)
            bc2_sb = asb.tile([128, B], f32, tag="bc2sb", name="bc2sb", bufs=1)
            nc.vector.tensor_copy(bc2_sb[:], bc2_ps[:])
            H0 = spool.tile([128, HK, B], f8, tag="H", name="H")
            S0 = spool.tile([128, HK, B], bf16d, tag="S", name="S")
            for h in range(HK):
                nc.vector.tensor_tensor(H0[:, h, :], hu_ps[h][:], bc2_sb[:], ALU.mult)
                nc.vector.tensor_tensor(S0[:, h, :], hu_ps[h][:], bc2_sb[:], ALU.mult)

        # ======== Phase 4: decoder ========
        dsb = top.enter_context(tc.tile_pool(name="dsb", bufs=2))
        with tc.tile_pool(name="gpsd", bufs=2, space="PSUM") as gps_d:
            Hp8, Sp = H0, S0
            for s in range(L):
                # prefetch the step's one-hot target mask (gpsimd queue)
                oh = dsb.tile([128, B], f32r, tag="oh", name="oh")
                nc.gpsimd.dma_start(oh[:], oneh_d[s])
                Hp8, Sp = lstm_step(gps_d, xdec, s, Hp8, Sp)
                # logitsT [V, B] in slice 0 of a rotating quad; Z and target
                # exp(logit) reductions land in spare banks 1 and 2
                lq = gps_d.tile([128, 4, B], f32, tag="quad", name="quad")
                nc.tensor.matmul(lq[:, 0, :], Wout8[:, 0:2, :], Hp8[:, 0:2, :],
                                 start=True, stop=False, perf_mode=DR)
                nc.tensor.matmul(lq[:, 0, :], Wout8[:, 2:4, :], Hp8[:, 2:4, :],
                                 start=False, stop=True, perf_mode=DR)
                El = dsb.tile([128, B], f32r, tag="El", name="El")
                nc.scalar.activation(El[:], lq[:, 0, :], AF.Exp, scale=1.0 / SF)
                nc.tensor.matmul(lq[0:1, 1, :], ones_col[:], El[:],
                                 start=True, stop=True)
                ztmp = dsb.tile([1, B], f32, tag="ztmp", name="ztmp")
                nc.vector.tensor_copy(ztmp[:], lq[0:1, 1, :])
                nc.sync.dma_start(Zs[s:s + 1, :], ztmp[:])
                # exp(target logit) via host one-hot * El (on the otherwise
                # idle gpsimd); ln() undoes the exp in the post-pass
                mk = dsb.tile([128, B], f32r, tag="mk", name="mk")
                nc.gpsimd.tensor_tensor(mk[:], oh[:], El[:], ALU.mult)
                nc.tensor.matmul(lq[0:1, 2, :], ones_col[:], mk[:],
                                 start=True, stop=True)
                lttmp = dsb.tile([1, B], f32r, tag="lttmp", name="lttmp")
                nc.vector.tensor_copy(lttmp[:], lq[0:1, 2, :])
                nc.sync.dma_start(lts[s:s + 1, :], lttmp[:])

            # ======== Phase 5: final loss ========
            lnZ = dsb.tile([L, B], f32r, tag="lnZ", name="lnZ")
            nc.scalar.activation(lnZ[:], Zs[:], AF.Ln)
            lnLt = dsb.tile([L, B], f32r, tag="lnLt", name="lnLt")
            nc.scalar.activation(lnLt[:], lts[:], AF.Ln)
            diff = dsb.tile([L, B], f32r, tag="diff", name="diff")
            nc.vector.tensor_tensor(diff[:], lnZ[:], lnLt[:], ALU.subtract)
            fq = gps_d.tile([128, 4, B], f32, tag="quad", name="quad")
            nc.tensor.matmul(fq[0:1, 0, :], ones16[:], diff[:], start=True, stop=True)
            loss_sb = dsb.tile([1, B], f32, tag="losssb", name="losssb")
            nc.vector.tensor_scalar(loss_sb[:], fq[0:1, 0, :], 1.0 / L, None,
                                    ALU.mult)
            nc.sync.dma_start(out_d, loss_sb[:])

    nc.compile()
    return nc


def _prep_inputs(visual_input, text_input, emb, W_ih, W_hh, b_ih, b_hh,
                 W_enc, b_enc, W_out, W_vis):
    import ml_dtypes
    bf = ml_dtypes.bfloat16
    f8n = ml_dtypes.float8_e4m3
    f = np.float32
    vis = np.asarray(visual_input, f)[:, 0, :]              # [4096, 2048]
    text = np.asarray(text_input)
    emb = np.asarray(emb, f)
    visT = np.ascontiguousarray(vis.T)                      # [2048, 4096]
    WvisT = np.ascontiguousarray(np.asarray(W_vis, f).T)    # [2048, 512]

    # x-part weights x64 with bias folded as row 50; g-gate block x2 so all
    # gates share the ACT scale 0.5/64
    b = np.asarray(b_ih, f) + np.asarray(b_hh, f)           # [2048]
    WihT = np.concatenate([np.asarray(W_ih, f).T, b[None, :]], axis=0) * SF
    WihT[:, 2 * H:3 * H] *= 2.0                             # [51, 2048]

    def pack8(WT):                                          # [512, M] -> [128, 4, M]
        M = WT.shape[1]
        return np.ascontiguousarray(
            WT.reshape(HK, 128, M).transpose(1, 0, 2)).astype(f8n)

    WhhT = (0.5 * SF) * np.asarray(W_hh, f).T               # [512, 2048]
    WhhT[:, 2 * H:3 * H] *= 2.0
    Whh8 = pack8(WhhT)
    Wenc8 = pack8((0.5 * SF) * np.asarray(W_enc, f).T)      # [512, 512]
    Wout8 = pack8((0.5 * SF) * np.asarray(W_out, f).T)      # [512, 128]
    benc = np.ascontiguousarray(np.asarray(b_enc, f).reshape(HK, 128).T)

    encx = emb[text.T]                                      # [16, 4096, 50]
    dec_ch = np.concatenate([np.zeros((text.shape[0], 1), text.dtype),
                             text[:, :-1]], axis=1)
    decx = emb[dec_ch.T]                                    # [16, 4096, 50]
    one_row = np.ones((L, 1, B_FULL), f)
    encxT = np.concatenate([encx.transpose(0, 2, 1), one_row], axis=1)
    decxT = np.concatenate([decx.transpose(0, 2, 1), one_row], axis=1)
    encxT = np.ascontiguousarray(encxT)                     # [16, 51, 4096]
    decxT = np.ascontiguousarray(decxT)
    tgt = np.ascontiguousarray(text.T.astype(f))            # [16, 4096]
    iota = np.arange(128, dtype=f).reshape(128, 1)
    # one-hot target masks [L, V, B_FULL]
    oneh = (tgt[:, None, :] == iota.reshape(1, 128, 1)).astype(f)

    in_maps = []
    for c in range(NCORES):
        sl = slice(c * B, (c + 1) * B)
        in_maps.append({
            "visT": np.ascontiguousarray(visT[:, sl]).reshape(VISK, 128, B).astype(bf),
            "WvisT": WvisT.reshape(VISK, 128, H).astype(bf),
            "WihT": WihT.astype(bf),
            "Whh8": Whh8,
            "encx": np.ascontiguousarray(encxT[:, :, sl]).astype(bf),
            "decx": np.ascontiguousarray(decxT[:, :, sl]).astype(bf),
            "Wenc8": Wenc8,
            "benc": benc,
            "Wout8": Wout8,
            "tgt": np.ascontiguousarray(tgt[:, sl]),
            "iota128": iota,
            "oneh": np.ascontiguousarray(oneh[:, :, sl]),
            "ones128": np.ones((128, 1), np.float32),
        })
    return in_maps


LAST_EXEC_TIME_NS = None


def kernel(**inputs):
    global LAST_EXEC_TIME_NS
    from concourse.bass_utils import run_bass_kernel_spmd

    if "nc" not in _CACHE:
        _CACHE["nc"] = _build()
    nc = _CACHE["nc"]
    in_maps = _prep_inputs(**inputs)

    trace = bool(int(os.environ.get("KERNEL_PROFILE", "0")))
    kw = {}
    if trace:
        _install_profile_hook()
        kw["trace"] = True
    res = run_bass_kernel_spmd(nc, in_maps, core_ids=list(range(NCORES)), **kw)
    LAST_EXEC_TIME_NS = res.exec_time_ns
    out = np.concatenate([res.results[c]["loss"][0] for c in range(NCORES)])
    return out.astype(np.float32)


def _install_profile_hook():
    """Optional NTFF profiling (dev only; used when KERNEL_PROFILE=1)."""
    import types, ctypes, contextlib
    try:
        import antenv
    except ImportError:
        return
    if getattr(antenv, "axon_hooks", None) is not None:
        return
    mod = types.ModuleType('antenv.axon_hooks')
    _store = [None]
    mod.set_axon_ntff_profile_hook = lambda h: _store.__setitem__(0, h)
    mod.get_axon_ntff_profile_hook = lambda: _store[0]
    sys.modules['antenv.axon_hooks'] = mod
    antenv.axon_hooks = mod
    try:
        lib = ctypes.CDLL('/opt/axon/libaxon_pjrt.so')
    except OSError:
        return
    if not hasattr(lib, 'axon_start_nrt_profile'):
        return
    lib.axon_start_nrt_profile.argtypes = [ctypes.POINTER(ctypes.c_int64),
                                           ctypes.c_size_t]
    lib.axon_start_nrt_profile.restype = ctypes.c_int64
    lib.axon_stop_nrt_profile.argtypes = [ctypes.c_char_p]
    lib.axon_stop_nrt_profile.restype = ctypes.c_int64

    @contextlib.contextmanager
    def _hook(output_dir, device_ids):
        import jax
        jax.devices()
        if device_ids:
            ids = (ctypes.c_int64 * len(device_ids))(*device_ids)
            rc = lib.axon_start_nrt_profile(ids, len(device_ids))
        else:
            rc = lib.axon_start_nrt_profile(None, 0)
        if rc != 0:
            raise RuntimeError(f"axon_start_nrt_profile rc={rc}")
        try:
            yield
        finally:
            n = lib.axon_stop_nrt_profile(str(output_dir).encode())
            print(f"profile: {n} ntff file(s) in {output_dir}", file=sys.stderr)

    mod.set_axon_ntff_profile_hook = mod.set_axon_ntff_profile_hook
    mod.set_axon_ntff_profile_hook(_hook)
    import concourse.bass_utils as bu
    bu.upload_artifacts = lambda tmpdir: "local://" + str(tmpdir)


# revision 22
# speedup vs baseline: 1.1591x; 1.0263x over previous
"""Trainium2 Bass kernel for nn_AttentionModelCharLevel.

Model (per reference): visual linear -> char-encoder LSTM -> linear+relu ->
cosine attention (softmax over batch dim) -> char-decoder LSTM -> per-sample
mean NLL over L steps.

Sharding: data-parallel over batch B=4096 across 8 cores (512 rows each).
The [B,B] attention needs every core to see all normalized visual rows, so
each core computes its vn shard (+transpose), AllGathers both, and streams
the gathered rows back through SBUF during the attention phase.

Key device-side conventions (v2, fp8 DoubleRow):
- The LSTM recurrence, encoder linear and decoder logits matmuls run in
  fp8e4 with perf_mode=DoubleRow (2 fp8 K-rows per PE cell): K=512
  contractions become 2 matmuls of logical K=256. Weights are pre-scaled
  x64 host-side (fp8 normal range) and the x1/64 is folded into the ACT
  scale that reads the PSUM.
- Hidden state is stored doubled (Ht = 2h) as a single [128, 4, B] fp8
  tile per step; slice [:, 2g:2g+2, :] is the DoubleRow rhs pair. Cell
  state St = 2c is a [128, 4, B] bf16 tile.
- sigmoid(z) = 0.5*tanh(z/2)+0.5; the g-gate's weights carry an extra x2
  so ALL gates share one ACT scale (0.5/64). The bias is folded into the
  x-part matmul as a 51st embedding row (x=1), so one ACT with no bias
  covers a whole 4-bank PSUM quad [128, 4, B] = the 4 gates of one hidden
  chunk -> 4 gate ACTs + 1 tanh(c) ACT per step instead of 20.
- The DVE tail chain per hidden chunk runs in bf16 (2x DVE rate):
      m2 = (tanh_i + 1) * tanh_g
      m1 = (tanh_f + 1) * S
      S' = 0.5*m1 + m2
      H' = (tanh_o + 1) * tanh(0.5*S')   (fp8 out)
- Softmax over the batch dim reduces to exp() only: sims are cosine
  similarities in [-1,1] and the softmax denominator is a positive
  per-column scale that the final row normalization of h cancels.
- Decoder log-softmax: logits kept in [V, B] layout; Z = ones^T exp(logits)
  and the target logit via a one-hot mask built from an iota column --
  both reduce over partitions with K=128 ones-matmuls into spare banks of
  the rotating PSUM quads. ln() deferred to a single post-pass.
"""
import os
import sys

sys.path.insert(0, '/opt/trn_rl_repo')

import numpy as np

B_FULL = 4096
NCORES = 8
B = B_FULL // NCORES          # 512 rows per core
H = 512
G = 4 * H                     # 2048
E = 50
EA = E + 1                    # embedding dim + bias row
L = 16
V = 128
VIS = 2048
HK = H // 128                 # 4 chunks of the hidden dim
GK = G // 128                 # 16 gate chunks
VISK = VIS // 128             # 16 chunks of the visual dim
BK = B // 128                 # 4 batch chunks per core
VCHUNKS = B_FULL // 128       # 32 chunks of the full batch
SF = 64.0                     # fp8 weight scale

_CACHE = {}


def _build():
    import concourse.bass as bass
    import concourse.tile as tile
    import concourse.mybir as mybir
    from concourse import bacc
    from concourse.masks import make_identity
    from contextlib import ExitStack

    dt = mybir.dt
    AF = mybir.ActivationFunctionType
    ALU = mybir.AluOpType
    DR = mybir.MatmulPerfMode.DoubleRow
    f32 = dt.float32
    f32r = dt.float32r
    bf16d = dt.bfloat16
    f8 = dt.float8e4
    LN2 = float(np.log(2.0))

    AP = bass.AP
    nc = bacc.Bacc("TRN2", target_bir_lowering=False, debug=False,
                   num_devices=NCORES)

    # ---- DRAM I/O ----
    visT_d = nc.dram_tensor("visT", [VISK, 128, B], bf16d, kind="ExternalInput").ap()
    WvisT_d = nc.dram_tensor("WvisT", [VISK, 128, H], bf16d, kind="ExternalInput").ap()
    WihT_d = nc.dram_tensor("WihT", [EA, G], bf16d, kind="ExternalInput").ap()
    Whh8_d = nc.dram_tensor("Whh8", [128, HK, G], f8, kind="ExternalInput").ap()
    encx_d = nc.dram_tensor("encx", [L, EA, B], bf16d, kind="ExternalInput").ap()
    decx_d = nc.dram_tensor("decx", [L, EA, B], bf16d, kind="ExternalInput").ap()
    Wenc8_d = nc.dram_tensor("Wenc8", [128, HK, H], f8, kind="ExternalInput").ap()
    benc_d = nc.dram_tensor("benc", [128, HK], f32, kind="ExternalInput").ap()
    Wout8_d = nc.dram_tensor("Wout8", [128, HK, V], f8, kind="ExternalInput").ap()
    tgt_d = nc.dram_tensor("tgt", [L, B], f32, kind="ExternalInput").ap()
    iota_d = nc.dram_tensor("iota128", [128, 1], f32, kind="ExternalInput").ap()
    oneh_d = nc.dram_tensor("oneh", [L, 128, B], f32r, kind="ExternalInput").ap()
    ones_d = nc.dram_tensor("ones128", [128, 1], f32r, kind="ExternalInput").ap()
    out_d = nc.dram_tensor("loss", [1, B], f32, kind="ExternalOutput").ap()

    with tile.TileContext(nc) as tc, ExitStack() as top:
        wpool = top.enter_context(tc.tile_pool(name="w", bufs=1))
        spool = top.enter_context(tc.tile_pool(name="state", bufs=2))
        dram = top.enter_context(tc.tile_pool(name="dram", bufs=1, space="DRAM"))

        # ---- persistent weights / constants ----
        Whh8 = wpool.tile([128, HK, G], f8, tag="whh8", name="whh8")
        nc.sync.dma_start(Whh8[:], Whh8_d)
        WihT2 = wpool.tile([64 + EA, G], bf16d, tag="wih", name="wih")
        nc.sync.dma_start(WihT2[:EA, :], WihT_d)
        nc.sync.dma_start(WihT2[64:64 + EA, :], WihT_d)
        Wenc8 = wpool.tile([128, HK, H], f8, tag="wenc8", name="wenc8")
        nc.sync.dma_start(Wenc8[:], Wenc8_d)
        benc = wpool.tile([128, HK], f32, tag="benc", name="benc")
        nc.sync.dma_start(benc[:], benc_d)
        Wout8 = wpool.tile([128, HK, V], f8, tag="wout8", name="wout8")
        nc.sync.dma_start(Wout8[:], Wout8_d)
        iota_c = wpool.tile([128, 1], f32, tag="iota", name="iota")
        nc.sync.dma_start(iota_c[:], iota_d)

        ones_col = wpool.tile([128, 1], f32r, tag="ones_col", name="ones_col")
        nc.sync.dma_start(ones_col[:], ones_d)
        ones_row = wpool.tile([1, 128], f32r, tag="ones_row", name="ones_row")
        nc.sync.dma_start(ones_row[:], ones_d.rearrange("p one -> one p"))
        ones16 = wpool.tile([L, 1], f32r, tag="ones16", name="ones16")
        nc.sync.dma_start(ones16[:], ones_d[:L])
        ident = wpool.tile([128, 128], bf16d, tag="ident", name="ident")
        make_identity(nc, ident[:])
        ln2_t = wpool.tile([1, 1], f32, tag="ln2", name="ln2")
        nc.vector.memset(ln2_t[:], LN2)

        # prefetch ALL scan inputs up front on the gpsimd DMA queue so the
        # LSTM scans never wait on the sync queue / collective window
        xenc = wpool.tile([64 + EA, L, B], bf16d, tag="xenc", name="xenc")
        xdec = wpool.tile([64 + EA, L, B], bf16d, tag="xdec", name="xdec")
        for s in range(L):
            nc.gpsimd.dma_start(xenc[:EA, s, :], encx_d[s])
            nc.gpsimd.dma_start(xenc[64:64 + EA, s, :], encx_d[s])
        for s in range(L):
            nc.gpsimd.dma_start(xdec[:EA, s, :], decx_d[s])
            nc.gpsimd.dma_start(xdec[64:64 + EA, s, :], decx_d[s])

        # AllGather buffers: vn blocks and vnT blocks (both bf16)
        ag_in = dram.tile([BK, 128, B], bf16d, name="ag_in")
        ag_out = dram.tile([NCORES, BK, 128, B], bf16d, addr_space="Shared", name="ag_out")
        agt_in = dram.tile([HK, 128, B], bf16d, name="agt_in")
        agt_out = dram.tile([NCORES, HK, 128, B], bf16d, addr_space="Shared", name="agt_out")

        # decoder per-step Z and target-logit rows
        zpool = top.enter_context(tc.tile_pool(name="zp", bufs=1))
        Zs = zpool.tile([L, B], f32, tag="Zs", name="Zs")
        lts = zpool.tile([L, B], f32r, tag="lts", name="lts")

        # ======== Phase 1: visual linear + row-normalize + transpose + AG ====
        with ExitStack() as ph:
            vsb = ph.enter_context(tc.tile_pool(name="vsb", bufs=3))
            vps = ph.enter_context(tc.tile_pool(name="vps", bufs=1, space="PSUM"))
            tps = ph.enter_context(tc.tile_pool(name="tps", bufs=2, space="PSUM"))
            vnpool = ph.enter_context(tc.tile_pool(name="vnp", bufs=1))

            v_ps = [vps.tile([128, H], f32, tag=f"vps{b}", name=f"vps{b}") for b in range(BK)]
            for ki in range(VISK):
                vis_t = vsb.tile([128, B], bf16d, tag="vis", name="vis")
                nc.sync.dma_start(vis_t[:], visT_d[ki])
                wv_t = vsb.tile([128, H], bf16d, tag="wvis", name="wvis")
                nc.sync.dma_start(wv_t[:], WvisT_d[ki])
                for b in range(BK):
                    nc.tensor.matmul(v_ps[b][:], vis_t[:, b * 128:(b + 1) * 128],
                                     wv_t[:], start=(ki == 0), stop=(ki == VISK - 1))
            vn = []
            for b in range(BK):
                sq = vsb.tile([128, H], f32, tag="vsq", name="vsq")
                s_col = vsb.tile([128, 1], f32, tag="vscol", name="vscol")
                nc.scalar.activation(sq[:], v_ps[b][:], AF.Square,
                                     accum_out=s_col[:])
                lnc_ = vsb.tile([128, 1], f32, tag="vln", name="vln")
                nc.scalar.activation(lnc_[:], s_col[:], AF.Ln)
                rs = vsb.tile([128, 1], f32, tag="vrs", name="vrs")
                nc.scalar.activation(rs[:], lnc_[:], AF.Exp, scale=-0.5)
                vn_b = vnpool.tile([128, H], bf16d, tag=f"vn{b}", name=f"vn{b}")
                nc.vector.tensor_scalar(vn_b[:], v_ps[b][:], rs[:], None, ALU.mult)
                vn.append(vn_b)
            # transpose vn -> vnT (16 128x128 blocks, bf16 for the sims lhsT)
            vnT = [vnpool.tile([128, B], bf16d, tag=f"vnT{h}", name=f"vnT{h}") for h in range(HK)]
            for b in range(BK):
                for h in range(HK):
                    t_ps = tps.tile([128, 128], bf16d, tag="tr", name="tr")
                    nc.tensor.transpose(
                        t_ps[:], vn[b][:, h * 128:(h + 1) * 128], ident[:])
                    nc.vector.tensor_copy(vnT[h][:, b * 128:(b + 1) * 128], t_ps[:])
            for b in range(BK):
                nc.sync.dma_start(ag_in[b], vn[b][:])
            for h in range(HK):
                nc.sync.dma_start(agt_in[h], vnT[h][:])
            nc.gpsimd.collective_compute(
                "AllGather", mybir.AluOpType.bypass,
                replica_groups=[list(range(NCORES))],
                ins=[ag_in[:]], outs=[ag_out[:]],
            )
            nc.gpsimd.collective_compute(
                "AllGather", mybir.AluOpType.bypass,
                replica_groups=[list(range(NCORES))],
                ins=[agt_in[:]], outs=[agt_out[:]],
            )

        # staged full vnT (bf16) for the attention sims lhsT; filled by
        # gpsimd DMAs that wait on the collective and run during the encoder
        vnT_all = [wpool.tile([128, B_FULL], bf16d, tag=f"vnTall{k}", name=f"vnTall{k}")
                   for k in range(HK)]
        for k in range(HK):
            for r in range(NCORES):
                nc.gpsimd.dma_start(vnT_all[k][:, r * B:(r + 1) * B], agt_out[r, k])

        # ======== LSTM scan helper ========
        gsb = top.enter_context(tc.tile_pool(name="gsb", bufs=3))
        msb = top.enter_context(tc.tile_pool(name="msb", bufs=2))

        from collections import deque

        def lstm_step(gps, xall, s, Hp8, Sp, max_open=2):
            """One LSTM step, fp8 DoubleRow recurrence, quad-bank PSUM.

            Per hidden chunk j one 4-bank PSUM quad holds gates i,f,g,o.
            open = x-part matmuls (K=51, two concurrent row groups) plus the
            h01 DoubleRow pair; close = h23 pair + one mega-ACT over the quad
            + the state chain split across gpsimd (m2, m1) and DVE (S', H'),
            with tanh(c) in chunk pairs on ACT. Closes lag opens so the PE
            streams the next chunk while the previous drains, and the first
            closed chunks feed the next step's opens.
            """
            Hn8 = spool.tile([128, HK, B], f8, tag="H", name="H")
            Sn = spool.tile([128, HK, B], bf16d, tag="S", name="S")
            Ts = {}
            quads = {}

            def xmms(j):
                quad = gps.tile([128, 4, B], f32, tag="quad", name="quad")
                quads[j] = quad
                for gate in range(4):
                    c = gate * 4 + j
                    r0 = 0 if gate % 2 == 0 else 64
                    nc.tensor.matmul(quad[:, gate, :],
                                     WihT2[r0:r0 + EA, c * 128:(c + 1) * 128],
                                     xall[r0:r0 + EA, s, :], start=True, stop=False)

            def dr(j, lo, hi):
                quad = quads[j]
                for gate in range(4):
                    c = gate * 4 + j
                    nc.tensor.matmul(quad[:, gate, :],
                                     Whh8[:, lo:hi, c * 128:(c + 1) * 128],
                                     Hp8[:, lo:hi, :], start=False,
                                     stop=(hi == HK), perf_mode=DR)

            def finishA(j):
                T = gsb.tile([128, 4, B], bf16d, tag="T", name="T")
                Ts[j] = T
                nc.scalar.activation(T[:], quads[j][:], AF.Tanh, scale=0.5 / SF)
                m2 = msb.tile([128, B], bf16d, tag="m2", name="m2")
                nc.vector.scalar_tensor_tensor(m2[:], T[:, 0, :], 1.0,
                                               T[:, 2, :], ALU.add, ALU.mult)
                m1 = msb.tile([128, B], bf16d, tag="m1", name="m1")
                nc.vector.scalar_tensor_tensor(m1[:], T[:, 1, :], 1.0,
                                               Sp[:, j, :], ALU.add, ALU.mult)
                nc.vector.scalar_tensor_tensor(Sn[:, j, :], m1[:], 0.5, m2[:],
                                               ALU.mult, ALU.add)

            def finishB(j):
                th = msb.tile([128, B], bf16d, tag="th", name="th")
                nc.scalar.activation(th[:], Sn[:, j, :], AF.Tanh, scale=0.5)
                nc.vector.scalar_tensor_tensor(Hn8[:, j, :], Ts[j][:, 3, :],
                                               1.0, th[:], ALU.add, ALU.mult)

            # x-matmuls of chunks 0/1 are H-independent: they run while the
            # previous step's tail chain drains, keeping the PE warm. Full
            # per-chunk closes start the ACT/DVE tails as early as possible;
            # th/H' lag one chunk so they never head-of-line block the next
            # chunk's quad ACT in the ScalarE FIFO.
            xmms(0)
            xmms(1)
            dr(0, 0, 2)
            dr(0, 2, 4)
            finishA(0)
            dr(1, 0, 2)
            dr(1, 2, 4)
            finishA(1)
            finishB(0)
            xmms(2)
            dr(2, 0, 2)
            dr(2, 2, 4)
            finishA(2)
            finishB(1)
            xmms(3)
            dr(3, 0, 2)
            dr(3, 2, 4)
            finishA(3)
            finishB(2)
            finishB(3)
            return Hn8, Sn

        # ======== Phase 2: encoder ========
        Hp8 = spool.tile([128, HK, B], f8, tag="H", name="H")
        Sp = spool.tile([128, HK, B], bf16d, tag="S", name="S")
        nc.vector.memset(Hp8[:], 0.2)
        nc.vector.memset(Sp[:], 0.2)
        with tc.tile_pool(name="gpse", bufs=2, space="PSUM") as gps_e:
            for s in range(L):
                Hp8, Sp = lstm_step(gps_e, xenc, s, Hp8, Sp)
        Henc = Hp8

        # ======== Phase 3: t path + attention ========
        with ExitStack() as ph:
            asb = ph.enter_context(tc.tile_pool(name="asb", bufs=2))
            aps = ph.enter_context(tc.tile_pool(name="aps", bufs=1, space="PSUM"))
            sps_pool = ph.enter_context(tc.tile_pool(name="sps", bufs=2, space="PSUM"))
            vstr = ph.enter_context(tc.tile_pool(name="vstr", bufs=3))

            # t = relu(Wenc' @ Henc + benc), column-normalized
            tr = []
            s_ps = aps.tile([1, B], f32, tag="tsum", name="tsum")
            for mi in range(HK):
                t_ps = sps_pool.tile([128, B], f32, tag="sims", name="sims")
                nc.tensor.matmul(t_ps[:], Wenc8[:, 0:2, mi * 128:(mi + 1) * 128],
                                 Henc[:, 0:2, :], start=True, stop=False,
                                 perf_mode=DR)
                nc.tensor.matmul(t_ps[:], Wenc8[:, 2:4, mi * 128:(mi + 1) * 128],
                                 Henc[:, 2:4, :], start=False, stop=True,
                                 perf_mode=DR)
                tr_mi = asb.tile([128, B], f32, tag=f"tr{mi}", name=f"tr{mi}", bufs=1)
                nc.scalar.activation(tr_mi[:], t_ps[:], AF.Relu, scale=1.0 / SF,
                                     bias=benc[:, mi:mi + 1])
                tr.append(tr_mi)
                sq = asb.tile([128, B], f32r, tag="tsq", name="tsq")
                nc.scalar.activation(sq[:], tr_mi[:], AF.Square)
                nc.tensor.matmul(s_ps[:], ones_col[:], sq[:],
                                 start=(mi == 0), stop=(mi == HK - 1))
            lnr = asb.tile([1, B], f32, tag="tlnr", name="tlnr")
            nc.scalar.activation(lnr[:], s_ps[:], AF.Ln)
            rs_r = asb.tile([1, B], f32r, tag="trs", name="trs")
            nc.scalar.activation(rs_r[:], lnr[:], AF.Exp, scale=-0.5)
            bc_ps = aps.tile([128, B], f32, tag="tbc", name="tbc")
            nc.tensor.matmul(bc_ps[:], ones_row[:], rs_r[:], start=True, stop=True)
            tnT = []
            for mi in range(HK):
                tn_mi = asb.tile([128, B], bf16d, tag=f"tn{mi}", name=f"tn{mi}", bufs=1)
                nc.vector.tensor_tensor(tn_mi[:], tr[mi][:], bc_ps[:], ALU.mult)
                tnT.append(tn_mi)

            # attention: stream the gathered vn/vnT, E=exp(sims), accumulate h
            hu_ps = [aps.tile([128, B], f32, tag=f"hu{h}", name=f"hu{h}") for h in range(HK)]
            for i in range(VCHUNKS):
                r, b = divmod(i, BK)
                vn_i = vstr.tile([128, B], bf16d, tag="vni", name="vni", bufs=4)
                nc.sync.dma_start(vn_i[:], ag_out[r, b])
                sim_ps = sps_pool.tile([128, B], f32, tag="sims", name="sims")
                for k in range(HK):
                    nc.tensor.matmul(sim_ps[:],
                                     vnT_all[k][:, i * 128:(i + 1) * 128],
                                     tnT[k][:],
                                     start=(k == 0), stop=(k == HK - 1))
                E_i = vstr.tile([128, B], bf16d, tag="E", name="E")
                nc.scalar.activation(E_i[:], sim_ps[:], AF.Exp)
                for h in range(HK):
                    nc.tensor.matmul(hu_ps[h][:],
                                     vn_i[:, h * 128:(h + 1) * 128], E_i[:],
                                     start=(i == 0), stop=(i == VCHUNKS - 1))
            # normalize h (x2 for the doubled-state convention) -> decoder init
            s2_ps = aps.tile([1, B], f32, tag="tsum", name="tsum")
            for h in range(HK):
                sq = asb.tile([128, B], f32r, tag="husq", name="husq")
                nc.scalar.activation(sq[:], hu_ps[h][:], AF.Square)
                nc.tensor.matmul(s2_ps[:], ones_col[:], sq[:],
                                 start=(h == 0), stop=(h == HK - 1))
            lnr2 = asb.tile([1, B], f32, tag="hulnr", name="hulnr")
            nc.scalar.activation(lnr2[:], s2_ps[:], AF.Ln)
            rs2 = asb.tile([1, B], f32r, tag="hurs", name="hurs")
            nc.scalar.activation(rs2[:], lnr2[:], AF.Exp, scale=-0.5, bias=ln2_t[:])
            bc2_ps = aps.tile([128, B], f32, tag="tbc", name="tbc")
            nc.tensor.matmul(bc2_ps[:], ones_row[:], rs2[:], start=True, stop=True)
            bc2_sb = asb.tile([128, B], f32, tag="bc2sb", name="bc2sb", bufs=1)
            nc.vector.tensor_copy(bc2_sb[:], bc2_ps[:])
            H0 = spool.tile([128, HK, B], f8, tag="H", name="H")
            S0 = spool.tile([128, HK, B], bf16d, tag="S", name="S")
            for h in range(HK):
                nc.vector.tensor_tensor(H0[:, h, :], hu_ps[h][:], bc2_sb[:], ALU.mult)
                nc.vector.tensor_tensor(S0[:, h, :], hu_ps[h][:], bc2_sb[:], ALU.mult)

        # ======== Phase 4: decoder ========
        dsb = top.enter_context(tc.tile_pool(name="dsb", bufs=2))
        with tc.tile_pool(name="gpsd", bufs=2, space="PSUM") as gps_d:
            Hp8, Sp = H0, S0
            for s in range(L):
                # prefetch the step's one-hot target mask (gpsimd queue)
                oh = dsb.tile([128, B], f32r, tag="oh", name="oh")
                nc.gpsimd.dma_start(oh[:], oneh_d[s])
                Hp8, Sp = lstm_step(gps_d, xdec, s, Hp8, Sp)
                # logitsT [V, B] in slice 0 of a rotating quad; Z and target
                # exp(logit) reductions land in spare banks 1 and 2
                lq = gps_d.tile([128, 4, B], f32, tag="quad", name="quad")
                nc.tensor.matmul(lq[:, 0, :], Wout8[:, 0:2, :], Hp8[:, 0:2, :],
                                 start=True, stop=False, perf_mode=DR)
                nc.tensor.matmul(lq[:, 0, :], Wout8[:, 2:4, :], Hp8[:, 2:4, :],
                                 start=False, stop=True, perf_mode=DR)
                El = dsb.tile([128, B], f32r, tag="El", name="El")
                nc.scalar.activation(El[:], lq[:, 0, :], AF.Exp, scale=1.0 / SF)
                nc.tensor.matmul(lq[0:1, 1, :], ones_col[:], El[:],
                                 start=True, stop=True)
                ztmp = dsb.tile([1, B], f32, tag="ztmp", name="ztmp")
                nc.vector.tensor_copy(ztmp[:], lq[0:1, 1, :])
                nc.sync.dma_start(Zs[s:s + 1, :], ztmp[:])
                # exp(target logit) via host one-hot * El (on the otherwise
                # idle gpsimd); ln() undoes the exp in the post-pass
                mk = dsb.tile([128, B], f32r, tag="mk", name="mk")
                nc.gpsimd.tensor_tensor(mk[:], oh[:], El[:], ALU.mult)
                nc.tensor.matmul(lq[0:1, 2, :], ones_col[:], mk[:],
                                 start=True, stop=True)
                lttmp = dsb.tile([1, B], f32r, tag="lttmp", name="lttmp")
                nc.vector.tensor_copy(lttmp[:], lq[0:1, 2, :])
                nc.sync.dma_start(lts[s:s + 1, :], lttmp[:])

            # ======== Phase 5: final loss ========
            lnZ = dsb.tile([L, B], f32r, tag="lnZ", name="lnZ")
            nc.scalar.activation(lnZ[:], Zs[:], AF.Ln)
            lnLt = dsb.tile([L, B], f32r, tag="lnLt", name="lnLt")
            nc.scalar.activation(lnLt[:], lts[:], AF.Ln)
            diff = dsb.tile([L, B], f32r, tag="diff", name="diff")
            nc.vector.tensor_tensor(diff[:], lnZ[:], lnLt[:], ALU.subtract)
            fq = gps_d.tile([128, 4, B], f32, tag="quad", name="quad")
            nc.tensor.matmul(fq[0:1, 0, :], ones16[:], diff[:], start=True, stop=True)
            loss_sb = dsb.tile([1, B], f32, tag="losssb", name="losssb")
            nc.vector.tensor_scalar(loss_sb[:], fq[0:1, 0, :], 1.0 / L, None,
                                    ALU.mult)
            nc.sync.dma_start(out_d, loss_sb[:])

    nc.compile()
    return nc


def _prep_inputs(visual_input, text_input, emb, W_ih, W_hh, b_ih, b_hh,
                 W_enc, b_enc, W_out, W_vis):
    import ml_dtypes
    bf = ml_dtypes.bfloat16
    f8n = ml_dtypes.float8_e4m3
    f = np.float32
    vis = np.asarray(visual_input, f)[:, 0, :]              # [4096, 2048]
    text = np.asarray(text_input)
    emb = np.asarray(emb, f)
    visT = np.ascontiguousarray(vis.T)                      # [2048, 4096]
    WvisT = np.ascontiguousarray(np.asarray(W_vis, f).T)    # [2048, 512]

    # x-part weights x64 with bias folded as row 50; g-gate block x2 so all
    # gates share the ACT scale 0.5/64
    b = np.asarray(b_ih, f) + np.asarray(b_hh, f)           # [2048]
    WihT = np.concatenate([np.asarray(W_ih, f).T, b[None, :]], axis=0) * SF
    WihT[:, 2 * H:3 * H] *= 2.0                             # [51, 2048]

    def pack8(WT):                                          # [512, M] -> [128, 4, M]
        M = WT.shape[1]
        return np.ascontiguousarray(
            WT.reshape(HK, 128, M).transpose(1, 0, 2)).astype(f8n)

    WhhT = (0.5 * SF) * np.asarray(W_hh, f).T               # [512, 2048]
    WhhT[:, 2 * H:3 * H] *= 2.0
    Whh8 = pack8(WhhT)
    Wenc8 = pack8((0.5 * SF) * np.asarray(W_enc, f).T)      # [512, 512]
    Wout8 = pack8((0.5 * SF) * np.asarray(W_out, f).T)      # [512, 128]
    benc = np.ascontiguousarray(np.asarray(b_enc, f).reshape(HK, 128).T)

    encx = emb[text.T]                                      # [16, 4096, 50]
    dec_ch = np.concatenate([np.zeros((text.shape[0], 1), text.dtype),
                             text[:, :-1]], axis=1)
    decx = emb[dec_ch.T]                                    # [16, 4096, 50]
    one_row = np.ones((L, 1, B_FULL), f)
    encxT = np.concatenate([encx.transpose(0, 2, 1), one_row], axis=1)
    decxT = np.concatenate([decx.transpose(0, 2, 1), one_row], axis=1)
    encxT = np.ascontiguousarray(encxT)                     # [16, 51, 4096]
    decxT = np.ascontiguousarray(decxT)
    tgt = np.ascontiguousarray(text.T.astype(f))            # [16, 4096]
    iota = np.arange(128, dtype=f).reshape(128, 1)
    # one-hot target masks [L, V, B_FULL]
    oneh = (tgt[:, None, :] == iota.reshape(1, 128, 1)).astype(f)

    in_maps = []
    for c in range(NCORES):
        sl = slice(c * B, (c + 1) * B)
        in_maps.append({
            "visT": np.ascontiguousarray(visT[:, sl]).reshape(VISK, 128, B).astype(bf),
            "WvisT": WvisT.reshape(VISK, 128, H).astype(bf),
            "WihT": WihT.astype(bf),
            "Whh8": Whh8,
            "encx": np.ascontiguousarray(encxT[:, :, sl]).astype(bf),
            "decx": np.ascontiguousarray(decxT[:, :, sl]).astype(bf),
            "Wenc8": Wenc8,
            "benc": benc,
            "Wout8": Wout8,
            "tgt": np.ascontiguousarray(tgt[:, sl]),
            "iota128": iota,
            "oneh": np.ascontiguousarray(oneh[:, :, sl]),
            "ones128": np.ones((128, 1), np.float32),
        })
    return in_maps


LAST_EXEC_TIME_NS = None


def kernel(**inputs):
    global LAST_EXEC_TIME_NS
    from concourse.bass_utils import run_bass_kernel_spmd

    if "nc" not in _CACHE:
        _CACHE["nc"] = _build()
    nc = _CACHE["nc"]
    in_maps = _prep_inputs(**inputs)

    trace = bool(int(os.environ.get("KERNEL_PROFILE", "0")))
    kw = {}
    if trace:
        _install_profile_hook()
        kw["trace"] = True
    res = run_bass_kernel_spmd(nc, in_maps, core_ids=list(range(NCORES)), **kw)
    LAST_EXEC_TIME_NS = res.exec_time_ns
    out = np.concatenate([res.results[c]["loss"][0] for c in range(NCORES)])
    return out.astype(np.float32)


def _install_profile_hook():
    """Optional NTFF profiling (dev only; used when KERNEL_PROFILE=1)."""
    import types, ctypes, contextlib
    try:
        import antenv
    except ImportError:
        return
    if getattr(antenv, "axon_hooks", None) is not None:
        return
    mod = types.ModuleType('antenv.axon_hooks')
    _store = [None]
    mod.set_axon_ntff_profile_hook = lambda h: _store.__setitem__(0, h)
    mod.get_axon_ntff_profile_hook = lambda: _store[0]
    sys.modules['antenv.axon_hooks'] = mod
    antenv.axon_hooks = mod
    try:
        lib = ctypes.CDLL('/opt/axon/libaxon_pjrt.so')
    except OSError:
        return
    if not hasattr(lib, 'axon_start_nrt_profile'):
        return
    lib.axon_start_nrt_profile.argtypes = [ctypes.POINTER(ctypes.c_int64),
                                           ctypes.c_size_t]
    lib.axon_start_nrt_profile.restype = ctypes.c_int64
    lib.axon_stop_nrt_profile.argtypes = [ctypes.c_char_p]
    lib.axon_stop_nrt_profile.restype = ctypes.c_int64

    @contextlib.contextmanager
    def _hook(output_dir, device_ids):
        import jax
        jax.devices()
        if device_ids:
            ids = (ctypes.c_int64 * len(device_ids))(*device_ids)
            rc = lib.axon_start_nrt_profile(ids, len(device_ids))
        else:
            rc = lib.axon_start_nrt_profile(None, 0)
        if rc != 0:
            raise RuntimeError(f"axon_start_nrt_profile rc={rc}")
        try:
            yield
        finally:
            n = lib.axon_stop_nrt_profile(str(output_dir).encode())
            print(f"profile: {n} ntff file(s) in {output_dir}", file=sys.stderr)

    mod.set_axon_ntff_profile_hook = mod.set_axon_ntff_profile_hook
    mod.set_axon_ntff_profile_hook(_hook)
    import concourse.bass_utils as bu
    bu.upload_artifacts = lambda tmpdir: "local://" + str(tmpdir)


# revision 28
# speedup vs baseline: 1.2674x; 1.0934x over previous
"""Trainium2 Bass kernel for nn_AttentionModelCharLevel.

Model (per reference): visual linear -> char-encoder LSTM -> linear+relu ->
cosine attention (softmax over batch dim) -> char-decoder LSTM -> per-sample
mean NLL over L steps.

Sharding: data-parallel over batch B=4096 across 8 cores (512 rows each).
The [B,B] attention needs every core to see all normalized visual rows, so
each core computes its vn shard (+transpose), AllGathers both, and streams
the gathered rows back through SBUF during the attention phase.

Key device-side conventions (v2, fp8 DoubleRow):
- The LSTM recurrence, encoder linear and decoder logits matmuls run in
  fp8e4 with perf_mode=DoubleRow (2 fp8 K-rows per PE cell): K=512
  contractions become 2 matmuls of logical K=256. Weights are pre-scaled
  x64 host-side (fp8 normal range) and the x1/64 is folded into the ACT
  scale that reads the PSUM.
- Hidden state is stored doubled (Ht = 2h) as a single [128, 4, B] fp8
  tile per step; slice [:, 2g:2g+2, :] is the DoubleRow rhs pair. Cell
  state St = 2c is a [128, 4, B] bf16 tile.
- sigmoid(z) = 0.5*tanh(z/2)+0.5; the g-gate's weights carry an extra x2
  so ALL gates share one ACT scale (0.5/64). The bias is folded into the
  x-part matmul as a 51st embedding row (x=1), so one ACT with no bias
  covers a whole 4-bank PSUM quad [128, 4, B] = the 4 gates of one hidden
  chunk -> 4 gate ACTs + 1 tanh(c) ACT per step instead of 20.
- The DVE tail chain per hidden chunk runs in bf16 (2x DVE rate):
      m2 = (tanh_i + 1) * tanh_g
      m1 = (tanh_f + 1) * S
      S' = 0.5*m1 + m2
      H' = (tanh_o + 1) * tanh(0.5*S')   (fp8 out)
- Softmax over the batch dim reduces to exp() only: sims are cosine
  similarities in [-1,1] and the softmax denominator is a positive
  per-column scale that the final row normalization of h cancels.
- Decoder log-softmax: logits kept in [V, B] layout; Z = ones^T exp(logits)
  and the target logit via a one-hot mask built from an iota column --
  both reduce over partitions with K=128 ones-matmuls into spare banks of
  the rotating PSUM quads. ln() deferred to a single post-pass.
"""
import os
import sys

sys.path.insert(0, '/opt/trn_rl_repo')

import numpy as np

B_FULL = 4096
NCORES = 8
B = B_FULL // NCORES          # 512 rows per core
H = 512
G = 4 * H                     # 2048
E = 50
EA = E + 1                    # embedding dim + bias row
L = 16
V = 128
VIS = 2048
HK = H // 128                 # 4 chunks of the hidden dim
GK = G // 128                 # 16 gate chunks
VISK = VIS // 128             # 16 chunks of the visual dim
BK = B // 128                 # 4 batch chunks per core
VCHUNKS = B_FULL // 128       # 32 chunks of the full batch
SF = 64.0                     # fp8 weight scale

_CACHE = {}


def _build():
    import concourse.bass as bass
    import concourse.tile as tile
    import concourse.mybir as mybir
    from concourse import bacc
    from concourse.masks import make_identity
    from contextlib import ExitStack

    dt = mybir.dt
    AF = mybir.ActivationFunctionType
    ALU = mybir.AluOpType
    DR = mybir.MatmulPerfMode.DoubleRow
    f32 = dt.float32
    f32r = dt.float32r
    bf16d = dt.bfloat16
    f8 = dt.float8e4
    LN2 = float(np.log(2.0))

    AP = bass.AP
    nc = bacc.Bacc("TRN2", target_bir_lowering=False, debug=False,
                   num_devices=NCORES)

    # ---- DRAM I/O ----
    visT_d = nc.dram_tensor("visT", [VISK, 128, B], bf16d, kind="ExternalInput").ap()
    WvisT_d = nc.dram_tensor("WvisT", [VISK, 128, H], bf16d, kind="ExternalInput").ap()
    WihT_d = nc.dram_tensor("WihT", [EA, G], bf16d, kind="ExternalInput").ap()
    Whh8_d = nc.dram_tensor("Whh8", [128, HK, G], f8, kind="ExternalInput").ap()
    encx_d = nc.dram_tensor("encx", [L, EA, B], bf16d, kind="ExternalInput").ap()
    decx_d = nc.dram_tensor("decx", [L, EA, B], bf16d, kind="ExternalInput").ap()
    Wenc8_d = nc.dram_tensor("Wenc8", [128, HK, H], f8, kind="ExternalInput").ap()
    benc_d = nc.dram_tensor("benc", [128, HK], f32, kind="ExternalInput").ap()
    Wout8_d = nc.dram_tensor("Wout8", [128, HK, V], f8, kind="ExternalInput").ap()
    tgt_d = nc.dram_tensor("tgt", [L, B], f32, kind="ExternalInput").ap()
    iota_d = nc.dram_tensor("iota128", [128, 1], f32, kind="ExternalInput").ap()
    oneh_d = nc.dram_tensor("oneh", [L, 128, B], f32r, kind="ExternalInput").ap()
    ones_d = nc.dram_tensor("ones128", [128, 1], f32r, kind="ExternalInput").ap()
    out_d = nc.dram_tensor("loss", [1, B], f32, kind="ExternalOutput").ap()

    with tile.TileContext(nc) as tc, ExitStack() as top:
        wpool = top.enter_context(tc.tile_pool(name="w", bufs=1))
        spool = top.enter_context(tc.tile_pool(name="state", bufs=2))
        dram = top.enter_context(tc.tile_pool(name="dram", bufs=1, space="DRAM"))

        # ---- persistent weights / constants ----
        Whh8 = wpool.tile([128, HK, G], f8, tag="whh8", name="whh8")
        nc.sync.dma_start(Whh8[:], Whh8_d)
        WihT2 = wpool.tile([64 + EA, G], bf16d, tag="wih", name="wih")
        nc.sync.dma_start(WihT2[:EA, :], WihT_d)
        nc.sync.dma_start(WihT2[64:64 + EA, :], WihT_d)
        Wenc8 = wpool.tile([128, HK, H], f8, tag="wenc8", name="wenc8")
        nc.sync.dma_start(Wenc8[:], Wenc8_d)
        benc = wpool.tile([128, HK], f32, tag="benc", name="benc")
        nc.sync.dma_start(benc[:], benc_d)
        Wout8 = wpool.tile([128, HK, V], f8, tag="wout8", name="wout8")
        nc.sync.dma_start(Wout8[:], Wout8_d)
        iota_c = wpool.tile([128, 1], f32, tag="iota", name="iota")
        nc.sync.dma_start(iota_c[:], iota_d)

        ones_col = wpool.tile([128, 1], f32r, tag="ones_col", name="ones_col")
        nc.sync.dma_start(ones_col[:], ones_d)
        ones_row = wpool.tile([1, 128], f32r, tag="ones_row", name="ones_row")
        nc.sync.dma_start(ones_row[:], ones_d.rearrange("p one -> one p"))
        ones16 = wpool.tile([L, 1], f32r, tag="ones16", name="ones16")
        nc.sync.dma_start(ones16[:], ones_d[:L])
        ident = wpool.tile([128, 128], bf16d, tag="ident", name="ident")
        make_identity(nc, ident[:])
        ln2_t = wpool.tile([1, 1], f32, tag="ln2", name="ln2")
        nc.vector.memset(ln2_t[:], LN2)

        # prefetch ALL scan inputs up front on the gpsimd DMA queue so the
        # LSTM scans never wait on the sync queue / collective window
        xenc = wpool.tile([64 + EA, L, B], bf16d, tag="xenc", name="xenc")
        xdec = wpool.tile([64 + EA, L, B], bf16d, tag="xdec", name="xdec")
        for s in range(L):
            nc.gpsimd.dma_start(xenc[:EA, s, :], encx_d[s])
            nc.gpsimd.dma_start(xenc[64:64 + EA, s, :], encx_d[s])
        for s in range(L):
            nc.gpsimd.dma_start(xdec[:EA, s, :], decx_d[s])
            nc.gpsimd.dma_start(xdec[64:64 + EA, s, :], decx_d[s])

        # AllGather buffers: vn blocks and vnT blocks (both bf16)
        ag_in = dram.tile([BK, 128, B], bf16d, name="ag_in")
        ag_out = dram.tile([NCORES, BK, 128, B], bf16d, addr_space="Shared", name="ag_out")
        agt_in = dram.tile([HK, 128, B], bf16d, name="agt_in")
        agt_out = dram.tile([NCORES, HK, 128, B], bf16d, addr_space="Shared", name="agt_out")

        # decoder per-step Z and target-logit rows
        zpool = top.enter_context(tc.tile_pool(name="zp", bufs=1))
        Zs = zpool.tile([L, B], f32, tag="Zs", name="Zs")
        lts = zpool.tile([L, B], f32r, tag="lts", name="lts")

        # ======== Phase 1: visual linear + row-normalize + transpose + AG ====
        with ExitStack() as ph:
            vsb = ph.enter_context(tc.tile_pool(name="vsb", bufs=3))
            vps = ph.enter_context(tc.tile_pool(name="vps", bufs=1, space="PSUM"))
            tps = ph.enter_context(tc.tile_pool(name="tps", bufs=2, space="PSUM"))
            vnpool = ph.enter_context(tc.tile_pool(name="vnp", bufs=1))

            v_ps = [vps.tile([128, H], f32, tag=f"vps{b}", name=f"vps{b}") for b in range(BK)]
            for ki in range(VISK):
                vis_t = vsb.tile([128, B], bf16d, tag="vis", name="vis")
                nc.sync.dma_start(vis_t[:], visT_d[ki])
                wv_t = vsb.tile([128, H], bf16d, tag="wvis", name="wvis")
                nc.sync.dma_start(wv_t[:], WvisT_d[ki])
                for b in range(BK):
                    nc.tensor.matmul(v_ps[b][:], vis_t[:, b * 128:(b + 1) * 128],
                                     wv_t[:], start=(ki == 0), stop=(ki == VISK - 1))
            vn = []
            for b in range(BK):
                sq = vsb.tile([128, H], f32, tag="vsq", name="vsq")
                s_col = vsb.tile([128, 1], f32, tag="vscol", name="vscol")
                nc.scalar.activation(sq[:], v_ps[b][:], AF.Square,
                                     accum_out=s_col[:])
                lnc_ = vsb.tile([128, 1], f32, tag="vln", name="vln")
                nc.scalar.activation(lnc_[:], s_col[:], AF.Ln)
                rs = vsb.tile([128, 1], f32, tag="vrs", name="vrs")
                nc.scalar.activation(rs[:], lnc_[:], AF.Exp, scale=-0.5)
                vn_b = vnpool.tile([128, H], bf16d, tag=f"vn{b}", name=f"vn{b}")
                nc.vector.tensor_scalar(vn_b[:], v_ps[b][:], rs[:], None, ALU.mult)
                vn.append(vn_b)
            # transpose vn -> vnT (16 128x128 blocks, bf16 for the sims lhsT)
            vnT = [vnpool.tile([128, B], bf16d, tag=f"vnT{h}", name=f"vnT{h}") for h in range(HK)]
            for b in range(BK):
                for h in range(HK):
                    t_ps = tps.tile([128, 128], bf16d, tag="tr", name="tr")
                    nc.tensor.transpose(
                        t_ps[:], vn[b][:, h * 128:(h + 1) * 128], ident[:])
                    nc.vector.tensor_copy(vnT[h][:, b * 128:(b + 1) * 128], t_ps[:])
            for b in range(BK):
                nc.sync.dma_start(ag_in[b], vn[b][:])
            for h in range(HK):
                nc.sync.dma_start(agt_in[h], vnT[h][:])
            nc.gpsimd.collective_compute(
                "AllGather", mybir.AluOpType.bypass,
                replica_groups=[list(range(NCORES))],
                ins=[ag_in[:]], outs=[ag_out[:]],
            )
            nc.gpsimd.collective_compute(
                "AllGather", mybir.AluOpType.bypass,
                replica_groups=[list(range(NCORES))],
                ins=[agt_in[:]], outs=[agt_out[:]],
            )

        # staged full vnT (bf16) for the attention sims lhsT; filled by
        # gpsimd DMAs that wait on the collective and run during the encoder
        vnT_all = [wpool.tile([128, B_FULL], bf16d, tag=f"vnTall{k}", name=f"vnTall{k}")
                   for k in range(HK)]
        for k in range(HK):
            for r in range(NCORES):
                nc.gpsimd.dma_start(vnT_all[k][:, r * B:(r + 1) * B], agt_out[r, k])

        # ======== LSTM scan helper ========
        gsb = top.enter_context(tc.tile_pool(name="gsb", bufs=3))
        msb = top.enter_context(tc.tile_pool(name="msb", bufs=2))

        from collections import deque

        def lstm_step(gps, xall, s, Hp8, Sp, max_open=2):
            """One LSTM step, fp8 DoubleRow recurrence, quad-bank PSUM.

            Per hidden chunk j one 4-bank PSUM quad holds gates i,f,g,o.
            open = x-part matmuls (K=51, two concurrent row groups) plus the
            h01 DoubleRow pair; close = h23 pair + one mega-ACT over the quad
            + the state chain split across gpsimd (m2, m1) and DVE (S', H'),
            with tanh(c) in chunk pairs on ACT. Closes lag opens so the PE
            streams the next chunk while the previous drains, and the first
            closed chunks feed the next step's opens.
            """
            Hn8 = spool.tile([128, HK, B], f8, tag="H", name="H")
            Sn = spool.tile([128, HK, B], bf16d, tag="S", name="S")
            Ts = {}
            pairs = {}

            def xmms(j, half):
                pt = gps.tile([128, 2, B], f32, tag="pair", name="pair")
                pairs[(j, half)] = pt
                for gi in range(2):
                    gate = half * 2 + gi
                    c = gate * 4 + j
                    r0 = 0 if gi == 0 else 64
                    nc.tensor.matmul(pt[:, gi, :],
                                     WihT2[r0:r0 + EA, c * 128:(c + 1) * 128],
                                     xall[r0:r0 + EA, s, :], start=True, stop=False)

            def dr(j, half, lo, hi):
                pt = pairs[(j, half)]
                for gi in range(2):
                    gate = half * 2 + gi
                    c = gate * 4 + j
                    nc.tensor.matmul(pt[:, gi, :],
                                     Whh8[:, lo:hi, c * 128:(c + 1) * 128],
                                     Hp8[:, lo:hi, :], start=False,
                                     stop=(hi == HK), perf_mode=DR)

            def acts(j):
                T = gsb.tile([128, 4, B], bf16d, tag="T", name="T")
                Ts[j] = T
                nc.scalar.activation(T[:, 0:2, :], pairs[(j, 0)][:],
                                     AF.Tanh, scale=0.5 / SF)
                nc.scalar.activation(T[:, 2:4, :], pairs[(j, 1)][:],
                                     AF.Tanh, scale=0.5 / SF)

            def chain(j):
                T = Ts[j]
                m2 = msb.tile([128, B], bf16d, tag="m2", name="m2")
                nc.vector.scalar_tensor_tensor(m2[:], T[:, 0, :], 1.0,
                                               T[:, 2, :], ALU.add, ALU.mult)
                m1 = msb.tile([128, B], bf16d, tag="m1", name="m1")
                nc.vector.scalar_tensor_tensor(m1[:], T[:, 1, :], 1.0,
                                               Sp[:, j, :], ALU.add, ALU.mult)
                nc.vector.scalar_tensor_tensor(Sn[:, j, :], m1[:], 0.5, m2[:],
                                               ALU.mult, ALU.add)

            def thH(jlo):  # tanh(c) for chunk pair (jlo, jlo+1), then both H'
                th = msb.tile([128, 2, B], bf16d, tag="th", name="th")
                nc.scalar.activation(th[:], Sn[:, jlo:jlo + 2, :],
                                     AF.Tanh, scale=0.5)
                for jj in (jlo, jlo + 1):
                    nc.vector.scalar_tensor_tensor(Hn8[:, jj, :],
                                                   Ts[jj][:, 3, :], 1.0,
                                                   th[:, jj - jlo, :],
                                                   ALU.add, ALU.mult)

            # 2-bank pair tiles, 4 in flight: two chunks of PE runway so the
            # ACT read latency never gaps the PE (keeps HAM warm). x-matmuls
            # of chunks 0/1 are H-independent and absorb the previous step's
            # tail; th/H' lag so they never head-of-line block quad ACTs.
            xmms(0, 0); xmms(0, 1); xmms(1, 0); xmms(1, 1)
            dr(0, 0, 0, 2); dr(0, 0, 2, 4); dr(0, 1, 0, 2); dr(0, 1, 2, 4)
            acts(0)
            dr(1, 0, 0, 2); dr(1, 0, 2, 4); dr(1, 1, 0, 2); dr(1, 1, 2, 4)
            acts(1)
            chain(0)
            xmms(2, 0); xmms(2, 1)
            dr(2, 0, 0, 2); dr(2, 0, 2, 4); dr(2, 1, 0, 2); dr(2, 1, 2, 4)
            acts(2)
            chain(1)
            thH(0)
            xmms(3, 0); xmms(3, 1)
            dr(3, 0, 0, 2); dr(3, 0, 2, 4); dr(3, 1, 0, 2); dr(3, 1, 2, 4)
            acts(3)
            chain(2)
            chain(3)
            thH(2)
            return Hn8, Sn

        # ======== Phase 2: encoder ========
        Hp8 = spool.tile([128, HK, B], f8, tag="H", name="H")
        Sp = spool.tile([128, HK, B], bf16d, tag="S", name="S")
        nc.vector.memset(Hp8[:], 0.2)
        nc.vector.memset(Sp[:], 0.2)
        with tc.tile_pool(name="gpse", bufs=4, space="PSUM") as gps_e:
            for s in range(L):
                Hp8, Sp = lstm_step(gps_e, xenc, s, Hp8, Sp)
        Henc = Hp8

        # ======== Phase 3: t path + attention ========
        with ExitStack() as ph:
            asb = ph.enter_context(tc.tile_pool(name="asb", bufs=2))
            aps = ph.enter_context(tc.tile_pool(name="aps", bufs=1, space="PSUM"))
            sps_pool = ph.enter_context(tc.tile_pool(name="sps", bufs=2, space="PSUM"))
            vstr = ph.enter_context(tc.tile_pool(name="vstr", bufs=3))

            # t = relu(Wenc' @ Henc + benc), column-normalized
            tr = []
            s_ps = aps.tile([1, B], f32, tag="tsum", name="tsum")
            for mi in range(HK):
                t_ps = sps_pool.tile([128, B], f32, tag="sims", name="sims")
                nc.tensor.matmul(t_ps[:], Wenc8[:, 0:2, mi * 128:(mi + 1) * 128],
                                 Henc[:, 0:2, :], start=True, stop=False,
                                 perf_mode=DR)
                nc.tensor.matmul(t_ps[:], Wenc8[:, 2:4, mi * 128:(mi + 1) * 128],
                                 Henc[:, 2:4, :], start=False, stop=True,
                                 perf_mode=DR)
                tr_mi = asb.tile([128, B], f32, tag=f"tr{mi}", name=f"tr{mi}", bufs=1)
                nc.scalar.activation(tr_mi[:], t_ps[:], AF.Relu, scale=1.0 / SF,
                                     bias=benc[:, mi:mi + 1])
                tr.append(tr_mi)
                sq = asb.tile([128, B], f32r, tag="tsq", name="tsq")
                nc.scalar.activation(sq[:], tr_mi[:], AF.Square)
                nc.tensor.matmul(s_ps[:], ones_col[:], sq[:],
                                 start=(mi == 0), stop=(mi == HK - 1))
            lnr = asb.tile([1, B], f32, tag="tlnr", name="tlnr")
            nc.scalar.activation(lnr[:], s_ps[:], AF.Ln)
            rs_r = asb.tile([1, B], f32r, tag="trs", name="trs")
            nc.scalar.activation(rs_r[:], lnr[:], AF.Exp, scale=-0.5)
            bc_ps = aps.tile([128, B], f32, tag="tbc", name="tbc")
            nc.tensor.matmul(bc_ps[:], ones_row[:], rs_r[:], start=True, stop=True)
            tnT = []
            for mi in range(HK):
                tn_mi = asb.tile([128, B], bf16d, tag=f"tn{mi}", name=f"tn{mi}", bufs=1)
                nc.vector.tensor_tensor(tn_mi[:], tr[mi][:], bc_ps[:], ALU.mult)
                tnT.append(tn_mi)

            # attention: stream the gathered vn/vnT, E=exp(sims), accumulate h
            hu_ps = [aps.tile([128, B], f32, tag=f"hu{h}", name=f"hu{h}") for h in range(HK)]
            for i in range(VCHUNKS):
                r, b = divmod(i, BK)
                vn_i = vstr.tile([128, B], bf16d, tag="vni", name="vni", bufs=4)
                nc.sync.dma_start(vn_i[:], ag_out[r, b])
                sim_ps = sps_pool.tile([128, B], f32, tag="sims", name="sims")
                for k in range(HK):
                    nc.tensor.matmul(sim_ps[:],
                                     vnT_all[k][:, i * 128:(i + 1) * 128],
                                     tnT[k][:],
                                     start=(k == 0), stop=(k == HK - 1))
                E_i = vstr.tile([128, B], bf16d, tag="E", name="E")
                nc.scalar.activation(E_i[:], sim_ps[:], AF.Exp)
                for h in range(HK):
                    nc.tensor.matmul(hu_ps[h][:],
                                     vn_i[:, h * 128:(h + 1) * 128], E_i[:],
                                     start=(i == 0), stop=(i == VCHUNKS - 1))
            # normalize h (x2 for the doubled-state convention) -> decoder init
            s2_ps = aps.tile([1, B], f32, tag="tsum", name="tsum")
            for h in range(HK):
                sq = asb.tile([128, B], f32r, tag="husq", name="husq")
                nc.scalar.activation(sq[:], hu_ps[h][:], AF.Square)
                nc.tensor.matmul(s2_ps[:], ones_col[:], sq[:],
                                 start=(h == 0), stop=(h == HK - 1))
            lnr2 = asb.tile([1, B], f32, tag="hulnr", name="hulnr")
            nc.scalar.activation(lnr2[:], s2_ps[:], AF.Ln)
            rs2 = asb.tile([1, B], f32r, tag="hurs", name="hurs")
            nc.scalar.activation(rs2[:], lnr2[:], AF.Exp, scale=-0.5, bias=ln2_t[:])
            bc2_ps = aps.tile([128, B], f32, tag="tbc", name="tbc")
            nc.tensor.matmul(bc2_ps[:], ones_row[:], rs2[:], start=True, stop=True)
            bc2_sb = asb.tile([128, B], f32, tag="bc2sb", name="bc2sb", bufs=1)
            nc.vector.tensor_copy(bc2_sb[:], bc2_ps[:])
            H0 = spool.tile([128, HK, B], f8, tag="H", name="H")
            S0 = spool.tile([128, HK, B], bf16d, tag="S", name="S")
            for h in range(HK):
                nc.vector.tensor_tensor(H0[:, h, :], hu_ps[h][:], bc2_sb[:], ALU.mult)
                nc.vector.tensor_tensor(S0[:, h, :], hu_ps[h][:], bc2_sb[:], ALU.mult)

        # ======== Phase 4: decoder ========
        dsb = top.enter_context(tc.tile_pool(name="dsb", bufs=2))
        with tc.tile_pool(name="gpsd", bufs=4, space="PSUM") as gps_d:
            Hp8, Sp = H0, S0
            for s in range(L):
                # prefetch the step's one-hot target mask (gpsimd queue)
                oh = dsb.tile([128, B], f32r, tag="oh", name="oh")
                nc.gpsimd.dma_start(oh[:], oneh_d[s])
                Hp8, Sp = lstm_step(gps_d, xdec, s, Hp8, Sp)
                # logitsT [V, B] in bank 0 of a rotating pair tile; Z and
                # target exp(logit) reductions land in bank 1 (partitions 0/32)
                lq = gps_d.tile([128, 2, B], f32, tag="pair", name="pair")
                nc.tensor.matmul(lq[:, 0, :], Wout8[:, 0:2, :], Hp8[:, 0:2, :],
                                 start=True, stop=False, perf_mode=DR)
                nc.tensor.matmul(lq[:, 0, :], Wout8[:, 2:4, :], Hp8[:, 2:4, :],
                                 start=False, stop=True, perf_mode=DR)
                El = dsb.tile([128, B], f32r, tag="El", name="El")
                nc.scalar.activation(El[:], lq[:, 0, :], AF.Exp, scale=1.0 / SF)
                nc.tensor.matmul(lq[0:1, 1, :], ones_col[:], El[:],
                                 start=True, stop=True)
                ztmp = dsb.tile([1, B], f32, tag="ztmp", name="ztmp")
                nc.vector.tensor_copy(ztmp[:], lq[0:1, 1, :])
                nc.sync.dma_start(Zs[s:s + 1, :], ztmp[:])
                # exp(target logit) via host one-hot * El (on the otherwise
                # idle gpsimd); ln() undoes the exp in the post-pass
                mk = dsb.tile([128, B], f32r, tag="mk", name="mk")
                nc.gpsimd.tensor_tensor(mk[:], oh[:], El[:], ALU.mult)
                nc.tensor.matmul(lq[0:1, 1, :], ones_col[:], mk[:],
                                 start=True, stop=True)
                lttmp = dsb.tile([1, B], f32r, tag="lttmp", name="lttmp")
                nc.vector.tensor_copy(lttmp[:], lq[0:1, 1, :])
                nc.sync.dma_start(lts[s:s + 1, :], lttmp[:])

            # ======== Phase 5: final loss ========
            lnZ = dsb.tile([L, B], f32r, tag="lnZ", name="lnZ")
            nc.scalar.activation(lnZ[:], Zs[:], AF.Ln)
            lnLt = dsb.tile([L, B], f32r, tag="lnLt", name="lnLt")
            nc.scalar.activation(lnLt[:], lts[:], AF.Ln)
            diff = dsb.tile([L, B], f32r, tag="diff", name="diff")
            nc.vector.tensor_tensor(diff[:], lnZ[:], lnLt[:], ALU.subtract)
            fq = gps_d.tile([128, 2, B], f32, tag="pair", name="pair")
            nc.tensor.matmul(fq[0:1, 0, :], ones16[:], diff[:], start=True, stop=True)
            loss_sb = dsb.tile([1, B], f32, tag="losssb", name="losssb")
            nc.vector.tensor_scalar(loss_sb[:], fq[0:1, 0, :], 1.0 / L, None,
                                    ALU.mult)
            nc.sync.dma_start(out_d, loss_sb[:])

    nc.compile()
    return nc


def _prep_inputs(visual_input, text_input, emb, W_ih, W_hh, b_ih, b_hh,
                 W_enc, b_enc, W_out, W_vis):
    import ml_dtypes
    bf = ml_dtypes.bfloat16
    f8n = ml_dtypes.float8_e4m3
    f = np.float32
    vis = np.asarray(visual_input, f)[:, 0, :]              # [4096, 2048]
    text = np.asarray(text_input)
    emb = np.asarray(emb, f)
    visT = np.ascontiguousarray(vis.T)                      # [2048, 4096]
    WvisT = np.ascontiguousarray(np.asarray(W_vis, f).T)    # [2048, 512]

    # x-part weights x64 with bias folded as row 50; g-gate block x2 so all
    # gates share the ACT scale 0.5/64
    b = np.asarray(b_ih, f) + np.asarray(b_hh, f)           # [2048]
    WihT = np.concatenate([np.asarray(W_ih, f).T, b[None, :]], axis=0) * SF
    WihT[:, 2 * H:3 * H] *= 2.0                             # [51, 2048]

    def pack8(WT):                                          # [512, M] -> [128, 4, M]
        M = WT.shape[1]
        return np.ascontiguousarray(
            WT.reshape(HK, 128, M).transpose(1, 0, 2)).astype(f8n)

    WhhT = (0.5 * SF) * np.asarray(W_hh, f).T               # [512, 2048]
    WhhT[:, 2 * H:3 * H] *= 2.0
    Whh8 = pack8(WhhT)
    Wenc8 = pack8((0.5 * SF) * np.asarray(W_enc, f).T)      # [512, 512]
    Wout8 = pack8((0.5 * SF) * np.asarray(W_out, f).T)      # [512, 128]
    benc = np.ascontiguousarray(np.asarray(b_enc, f).reshape(HK, 128).T)

    encx = emb[text.T]                                      # [16, 4096, 50]
    dec_ch = np.concatenate([np.zeros((text.shape[0], 1), text.dtype),
                             text[:, :-1]], axis=1)
    decx = emb[dec_ch.T]                                    # [16, 4096, 50]
    one_row = np.ones((L, 1, B_FULL), f)
    encxT = np.concatenate([encx.transpose(0, 2, 1), one_row], axis=1)
    decxT = np.concatenate([decx.transpose(0, 2, 1), one_row], axis=1)
    encxT = np.ascontiguousarray(encxT)                     # [16, 51, 4096]
    decxT = np.ascontiguousarray(decxT)
    tgt = np.ascontiguousarray(text.T.astype(f))            # [16, 4096]
    iota = np.arange(128, dtype=f).reshape(128, 1)
    # one-hot target masks [L, V, B_FULL]
    oneh = (tgt[:, None, :] == iota.reshape(1, 128, 1)).astype(f)

    in_maps = []
    for c in range(NCORES):
        sl = slice(c * B, (c + 1) * B)
        in_maps.append({
            "visT": np.ascontiguousarray(visT[:, sl]).reshape(VISK, 128, B).astype(bf),
            "WvisT": WvisT.reshape(VISK, 128, H).astype(bf),
            "WihT": WihT.astype(bf),
            "Whh8": Whh8,
            "encx": np.ascontiguousarray(encxT[:, :, sl]).astype(bf),
            "decx": np.ascontiguousarray(decxT[:, :, sl]).astype(bf),
            "Wenc8": Wenc8,
            "benc": benc,
            "Wout8": Wout8,
            "tgt": np.ascontiguousarray(tgt[:, sl]),
            "iota128": iota,
            "oneh": np.ascontiguousarray(oneh[:, :, sl]),
            "ones128": np.ones((128, 1), np.float32),
        })
    return in_maps


LAST_EXEC_TIME_NS = None


def kernel(**inputs):
    global LAST_EXEC_TIME_NS
    from concourse.bass_utils import run_bass_kernel_spmd

    if "nc" not in _CACHE:
        _CACHE["nc"] = _build()
    nc = _CACHE["nc"]
    in_maps = _prep_inputs(**inputs)

    trace = bool(int(os.environ.get("KERNEL_PROFILE", "0")))
    kw = {}
    if trace:
        _install_profile_hook()
        kw["trace"] = True
    res = run_bass_kernel_spmd(nc, in_maps, core_ids=list(range(NCORES)), **kw)
    LAST_EXEC_TIME_NS = res.exec_time_ns
    out = np.concatenate([res.results[c]["loss"][0] for c in range(NCORES)])
    return out.astype(np.float32)


def _install_profile_hook():
    """Optional NTFF profiling (dev only; used when KERNEL_PROFILE=1)."""
    import types, ctypes, contextlib
    try:
        import antenv
    except ImportError:
        return
    if getattr(antenv, "axon_hooks", None) is not None:
        return
    mod = types.ModuleType('antenv.axon_hooks')
    _store = [None]
    mod.set_axon_ntff_profile_hook = lambda h: _store.__setitem__(0, h)
    mod.get_axon_ntff_profile_hook = lambda: _store[0]
    sys.modules['antenv.axon_hooks'] = mod
    antenv.axon_hooks = mod
    try:
        lib = ctypes.CDLL('/opt/axon/libaxon_pjrt.so')
    except OSError:
        return
    if not hasattr(lib, 'axon_start_nrt_profile'):
        return
    lib.axon_start_nrt_profile.argtypes = [ctypes.POINTER(ctypes.c_int64),
                                           ctypes.c_size_t]
    lib.axon_start_nrt_profile.restype = ctypes.c_int64
    lib.axon_stop_nrt_profile.argtypes = [ctypes.c_char_p]
    lib.axon_stop_nrt_profile.restype = ctypes.c_int64

    @contextlib.contextmanager
    def _hook(output_dir, device_ids):
        import jax
        jax.devices()
        if device_ids:
            ids = (ctypes.c_int64 * len(device_ids))(*device_ids)
            rc = lib.axon_start_nrt_profile(ids, len(device_ids))
        else:
            rc = lib.axon_start_nrt_profile(None, 0)
        if rc != 0:
            raise RuntimeError(f"axon_start_nrt_profile rc={rc}")
        try:
            yield
        finally:
            n = lib.axon_stop_nrt_profile(str(output_dir).encode())
            print(f"profile: {n} ntff file(s) in {output_dir}", file=sys.stderr)

    mod.set_axon_ntff_profile_hook = mod.set_axon_ntff_profile_hook
    mod.set_axon_ntff_profile_hook(_hook)
    import concourse.bass_utils as bu
    bu.upload_artifacts = lambda tmpdir: "local://" + str(tmpdir)


# revision 42
# speedup vs baseline: 1.2925x; 1.0198x over previous
"""Trainium2 Bass kernel for nn_AttentionModelCharLevel.

Model (per reference): visual linear -> char-encoder LSTM -> linear+relu ->
cosine attention (softmax over batch dim) -> char-decoder LSTM -> per-sample
mean NLL over L steps.

Sharding: data-parallel over batch B=4096 across 8 cores (512 rows each).
The [B,B] attention needs every core to see all normalized visual rows, so
each core computes its vn shard (+transpose), AllGathers both, and streams
the gathered rows back through SBUF during the attention phase.

Key device-side conventions (v2, fp8 DoubleRow):
- The LSTM recurrence, encoder linear and decoder logits matmuls run in
  fp8e4 with perf_mode=DoubleRow (2 fp8 K-rows per PE cell): K=512
  contractions become 2 matmuls of logical K=256. Weights are pre-scaled
  x64 host-side (fp8 normal range) and the x1/64 is folded into the ACT
  scale that reads the PSUM.
- Hidden state is stored doubled (Ht = 2h) as a single [128, 4, B] fp8
  tile per step; slice [:, 2g:2g+2, :] is the DoubleRow rhs pair. Cell
  state St = 2c is a [128, 4, B] bf16 tile.
- sigmoid(z) = 0.5*tanh(z/2)+0.5; the g-gate's weights carry an extra x2
  so ALL gates share one ACT scale (0.5/64). The bias is folded into the
  x-part matmul as a 51st embedding row (x=1), so one ACT with no bias
  covers a whole 4-bank PSUM quad [128, 4, B] = the 4 gates of one hidden
  chunk -> 4 gate ACTs + 1 tanh(c) ACT per step instead of 20.
- The DVE tail chain per hidden chunk runs in bf16 (2x DVE rate):
      m2 = (tanh_i + 1) * tanh_g
      m1 = (tanh_f + 1) * S
      S' = 0.5*m1 + m2
      H' = (tanh_o + 1) * tanh(0.5*S')   (fp8 out)
- Softmax over the batch dim reduces to exp() only: sims are cosine
  similarities in [-1,1] and the softmax denominator is a positive
  per-column scale that the final row normalization of h cancels.
- Decoder log-softmax: logits kept in [V, B] layout; Z = ones^T exp(logits)
  and the target logit via a one-hot mask built from an iota column --
  both reduce over partitions with K=128 ones-matmuls into spare banks of
  the rotating PSUM quads. ln() deferred to a single post-pass.
"""
import os
import sys

sys.path.insert(0, '/opt/trn_rl_repo')

import numpy as np

B_FULL = 4096
NCORES = 8
B = B_FULL // NCORES          # 512 rows per core
H = 512
G = 4 * H                     # 2048
E = 50
EA = E + 1                    # embedding dim + bias row
L = 16
V = 128
VIS = 2048
HK = H // 128                 # 4 chunks of the hidden dim
GK = G // 128                 # 16 gate chunks
VISK = VIS // 128             # 16 chunks of the visual dim
BK = B // 128                 # 4 batch chunks per core
VCHUNKS = B_FULL // 128       # 32 chunks of the full batch
SF = 64.0                     # fp8 weight scale

_CACHE = {}


def _build():
    import concourse.bass as bass
    import concourse.tile as tile
    import concourse.mybir as mybir
    from concourse import bacc
    from concourse.masks import make_identity
    from contextlib import ExitStack

    dt = mybir.dt
    AF = mybir.ActivationFunctionType
    ALU = mybir.AluOpType
    DR = mybir.MatmulPerfMode.DoubleRow
    f32 = dt.float32
    f32r = dt.float32r
    bf16d = dt.bfloat16
    f8 = dt.float8e4
    LN2 = float(np.log(2.0))

    AP = bass.AP
    nc = bacc.Bacc("TRN2", target_bir_lowering=False, debug=False,
                   num_devices=NCORES)

    # ---- DRAM I/O ----
    visT_d = nc.dram_tensor("visT", [VISK // 2, 128, 2, B], f8, kind="ExternalInput").ap()
    WvisT_d = nc.dram_tensor("WvisT", [VISK // 2, 128, 2, H], f8, kind="ExternalInput").ap()
    WihT_d = nc.dram_tensor("WihT", [EA, G], bf16d, kind="ExternalInput").ap()
    Whh8_d = nc.dram_tensor("Whh8", [128, HK, G], f8, kind="ExternalInput").ap()
    encx_d = nc.dram_tensor("encx", [L, EA, B], bf16d, kind="ExternalInput").ap()
    decx_d = nc.dram_tensor("decx", [L, EA, B], bf16d, kind="ExternalInput").ap()
    Wenc8_d = nc.dram_tensor("Wenc8", [128, HK, H], f8, kind="ExternalInput").ap()
    benc_d = nc.dram_tensor("benc", [128, HK], f32, kind="ExternalInput").ap()
    Wout8_d = nc.dram_tensor("Wout8", [128, HK, V], f8, kind="ExternalInput").ap()
    tgt_d = nc.dram_tensor("tgt", [L, B], f32, kind="ExternalInput").ap()
    iota_d = nc.dram_tensor("iota128", [128, 1], f32, kind="ExternalInput").ap()
    oneh_d = nc.dram_tensor("oneh", [L, 128, B], f32r, kind="ExternalInput").ap()
    ones_d = nc.dram_tensor("ones128", [128, 1], f32r, kind="ExternalInput").ap()
    out_d = nc.dram_tensor("loss", [1, B], f32, kind="ExternalOutput").ap()

    with tile.TileContext(nc) as tc, ExitStack() as top:
        wpool = top.enter_context(tc.tile_pool(name="w", bufs=1))
        spool = top.enter_context(tc.tile_pool(name="state", bufs=2))
        dram = top.enter_context(tc.tile_pool(name="dram", bufs=1, space="DRAM"))

        # ---- persistent weights / constants ----
        Whh8 = wpool.tile([128, HK, G], f8, tag="whh8", name="whh8")
        nc.sync.dma_start(Whh8[:], Whh8_d)
        WihT2 = wpool.tile([64 + EA, G], bf16d, tag="wih", name="wih")
        nc.sync.dma_start(WihT2[:EA, :], WihT_d)
        nc.sync.dma_start(WihT2[64:64 + EA, :], WihT_d)
        Wenc8 = wpool.tile([128, HK, H], f8, tag="wenc8", name="wenc8")
        nc.sync.dma_start(Wenc8[:], Wenc8_d)
        benc = wpool.tile([128, HK], f32, tag="benc", name="benc")
        nc.sync.dma_start(benc[:], benc_d)
        Wout8 = wpool.tile([128, HK, V], f8, tag="wout8", name="wout8")
        nc.sync.dma_start(Wout8[:], Wout8_d)
        iota_c = wpool.tile([128, 1], f32, tag="iota", name="iota")
        nc.sync.dma_start(iota_c[:], iota_d)

        ones_col = wpool.tile([128, 1], f32r, tag="ones_col", name="ones_col")
        nc.sync.dma_start(ones_col[:], ones_d)
        ones_row = wpool.tile([1, 128], f32r, tag="ones_row", name="ones_row")
        nc.sync.dma_start(ones_row[:], ones_d.rearrange("p one -> one p"))
        ones16 = wpool.tile([L, 1], f32r, tag="ones16", name="ones16")
        nc.sync.dma_start(ones16[:], ones_d[:L])
        ident8 = wpool.tile([128, 128], f8, tag="ident8", name="ident8")
        make_identity(nc, ident8[:])
        ln2_t = wpool.tile([1, 1], f32, tag="ln2", name="ln2")
        nc.vector.memset(ln2_t[:], LN2)
        ln16_t = wpool.tile([1, 1], f32, tag="ln16", name="ln16")
        nc.vector.memset(ln16_t[:], float(np.log(16.0)))
        ln16c = wpool.tile([128, 1], f32, tag="ln16c", name="ln16c")
        nc.vector.memset(ln16c[:], float(np.log(16.0)))

        # prefetch ALL scan inputs up front on the gpsimd DMA queue so the
        # LSTM scans never wait on the sync queue / collective window
        xenc = wpool.tile([64 + EA, L, B], bf16d, tag="xenc", name="xenc")
        xdec = wpool.tile([64 + EA, L, B], bf16d, tag="xdec", name="xdec")
        for s in range(L):
            nc.gpsimd.dma_start(xenc[:EA, s, :], encx_d[s])
            nc.gpsimd.dma_start(xenc[64:64 + EA, s, :], encx_d[s])
        for s in range(L):
            nc.gpsimd.dma_start(xdec[:EA, s, :], decx_d[s])
            nc.gpsimd.dma_start(xdec[64:64 + EA, s, :], decx_d[s])

        # AllGather buffers: vn blocks and vnT blocks (both fp8, x16 scale)
        ag_in = dram.tile([BK, 128, B], f8, name="ag_in")
        ag_out = dram.tile([NCORES, BK, 128, B], f8, addr_space="Shared", name="ag_out")
        agt_in = dram.tile([HK, 128, B], f8, name="agt_in")
        agt_out = dram.tile([NCORES, HK, 128, B], f8, addr_space="Shared", name="agt_out")

        # decoder per-step Z and target-logit rows
        zpool = top.enter_context(tc.tile_pool(name="zp", bufs=1))
        Zs = zpool.tile([L, B], f32, tag="Zs", name="Zs")
        lts = zpool.tile([L, B], f32r, tag="lts", name="lts")

        # ======== Phase 1: visual linear + row-normalize + transpose + AG ====
        with ExitStack() as ph:
            vsb = ph.enter_context(tc.tile_pool(name="vsb", bufs=3))
            vps = ph.enter_context(tc.tile_pool(name="vps", bufs=1, space="PSUM"))
            tps = ph.enter_context(tc.tile_pool(name="tps", bufs=2, space="PSUM"))
            vnpool = ph.enter_context(tc.tile_pool(name="vnp", bufs=1))

            v_ps = [vps.tile([128, H], f32, tag=f"vps{b}", name=f"vps{b}") for b in range(BK)]
            for ki in range(VISK // 2):
                vis_t = vsb.tile([128, 2, B], f8, tag="vis", name="vis")
                nc.sync.dma_start(vis_t[:], visT_d[ki])
                wv_t = vsb.tile([128, 2, H], f8, tag="wvis", name="wvis")
                nc.sync.dma_start(wv_t[:], WvisT_d[ki])
                for b in range(BK):
                    nc.tensor.matmul(v_ps[b][:], vis_t[:, :, b * 128:(b + 1) * 128],
                                     wv_t[:], start=(ki == 0),
                                     stop=(ki == VISK // 2 - 1), perf_mode=DR)
            vn = []
            for b in range(BK):
                sq = vsb.tile([128, H], f32, tag="vsq", name="vsq")
                s_col = vsb.tile([128, 1], f32, tag="vscol", name="vscol")
                nc.scalar.activation(sq[:], v_ps[b][:], AF.Square,
                                     accum_out=s_col[:])
                lnc_ = vsb.tile([128, 1], f32, tag="vln", name="vln")
                nc.scalar.activation(lnc_[:], s_col[:], AF.Ln)
                # sqrt(s) = 64*|v|, so exp(-0.5*ln s + ln16) puts vn in fp8
                # at x16 scale
                rs = vsb.tile([128, 1], f32, tag="vrs", name="vrs")
                nc.scalar.activation(rs[:], lnc_[:], AF.Exp, scale=-0.5,
                                     bias=ln16c[:])
                vn_b = vnpool.tile([128, H], f8, tag=f"vn{b}", name=f"vn{b}")
                nc.vector.tensor_scalar(vn_b[:], v_ps[b][:], rs[:], None, ALU.mult)
                vn.append(vn_b)
            # transpose vn -> vnT (16 128x128 blocks, fp8 for the sims lhsT)
            vnT = [vnpool.tile([128, B], f8, tag=f"vnT{h}", name=f"vnT{h}") for h in range(HK)]
            for b in range(BK):
                for h in range(HK):
                    t_ps = tps.tile([128, 128, 2], f8, tag="tr", name="tr")
                    nc.tensor.transpose(
                        t_ps[:, :, 0], vn[b][:, h * 128:(h + 1) * 128], ident8[:])
                    nc.vector.tensor_copy(vnT[h][:, b * 128:(b + 1) * 128],
                                          t_ps[:, :, 0])
            for b in range(BK):
                nc.sync.dma_start(ag_in[b], vn[b][:])
            for h in range(HK):
                nc.sync.dma_start(agt_in[h], vnT[h][:])
            nc.gpsimd.collective_compute(
                "AllGather", mybir.AluOpType.bypass,
                replica_groups=[list(range(NCORES))],
                ins=[ag_in[:]], outs=[ag_out[:]],
            )
            nc.gpsimd.collective_compute(
                "AllGather", mybir.AluOpType.bypass,
                replica_groups=[list(range(NCORES))],
                ins=[agt_in[:]], outs=[agt_out[:]],
            )

        # staged full vnT (fp8, DoubleRow pair layout) for the sims lhsT;
        # filled by gpsimd DMAs that wait on the collective during the encoder
        vnTp = [wpool.tile([128, 2, B_FULL], f8, tag=f"vnTp{g}", name=f"vnTp{g}")
                for g in range(2)]
        for k in range(HK):
            for r in range(NCORES):
                nc.gpsimd.dma_start(vnTp[k // 2][:, k % 2, r * B:(r + 1) * B],
                                    agt_out[r, k])

        # ======== LSTM scan helper ========
        gsb = top.enter_context(tc.tile_pool(name="gsb", bufs=3))
        msb = top.enter_context(tc.tile_pool(name="msb", bufs=2))

        from collections import deque

        def lstm_step(gps, xall, s, Hp8, Sp, max_open=2):
            """One LSTM step, fp8 DoubleRow recurrence, quad-bank PSUM.

            Per hidden chunk j one 4-bank PSUM quad holds gates i,f,g,o.
            open = x-part matmuls (K=51, two concurrent row groups) plus the
            h01 DoubleRow pair; close = h23 pair + one mega-ACT over the quad
            + the state chain split across gpsimd (m2, m1) and DVE (S', H'),
            with tanh(c) in chunk pairs on ACT. Closes lag opens so the PE
            streams the next chunk while the previous drains, and the first
            closed chunks feed the next step's opens.
            """
            Hn8 = spool.tile([128, HK, B], f8, tag="H", name="H")
            Sn = spool.tile([128, HK, B], bf16d, tag="S", name="S")
            Ts = {}
            pairs = {}

            def xmms(j, half):
                pt = gps.tile([128, 2, B], f32, tag="pair", name="pair")
                pairs[(j, half)] = pt
                for gi in range(2):
                    gate = half * 2 + gi
                    c = gate * 4 + j
                    r0 = 0 if gi == 0 else 64
                    nc.tensor.matmul(pt[:, gi, :],
                                     WihT2[r0:r0 + EA, c * 128:(c + 1) * 128],
                                     xall[r0:r0 + EA, s, :], start=True, stop=False)

            def dr(j, half, lo, hi):
                pt = pairs[(j, half)]
                for gi in range(2):
                    gate = half * 2 + gi
                    c = gate * 4 + j
                    nc.tensor.matmul(pt[:, gi, :],
                                     Whh8[:, lo:hi, c * 128:(c + 1) * 128],
                                     Hp8[:, lo:hi, :], start=False,
                                     stop=(hi == HK), perf_mode=DR)

            def acts(j):
                # T layout [128, gate, chunk%2, B]: each gate's two chunks
                # are contiguous, so the whole chunk-pair tail chain runs as
                # four [128, 2, B] DVE ops instead of eight chunk ops
                T = Ts[j // 2 * 2]
                if T is None:
                    T = gsb.tile([128, 4, 2, B], bf16d, tag="T", name="T")
                    Ts[j // 2 * 2] = T
                c = j % 2
                nc.scalar.activation(T[:, 0:2, c, :], pairs[(j, 0)][:],
                                     AF.Tanh, scale=0.5 / SF)
                nc.scalar.activation(T[:, 2:4, c, :], pairs[(j, 1)][:],
                                     AF.Tanh, scale=0.5 / SF)

            def chainP(jlo):  # m1/m2/S' for chunk pair (jlo, jlo+1)
                T = Ts[jlo]
                m1 = msb.tile([128, 2, B], bf16d, tag="m1", name="m1")
                nc.vector.scalar_tensor_tensor(m1[:], T[:, 1, :, :], 1.0,
                                               Sp[:, jlo:jlo + 2, :],
                                               ALU.add, ALU.mult)
                m2 = msb.tile([128, 2, B], bf16d, tag="m2", name="m2")
                nc.vector.scalar_tensor_tensor(m2[:], T[:, 0, :, :], 1.0,
                                               T[:, 2, :, :], ALU.add, ALU.mult)
                nc.vector.scalar_tensor_tensor(Sn[:, jlo:jlo + 2, :], m1[:],
                                               0.5, m2[:], ALU.mult, ALU.add)

            def thH(jlo):  # tanh(c) for the chunk pair, then both H'
                th = msb.tile([128, 2, B], bf16d, tag="th", name="th")
                nc.scalar.activation(th[:], Sn[:, jlo:jlo + 2, :],
                                     AF.Tanh, scale=0.5)
                nc.vector.scalar_tensor_tensor(Hn8[:, jlo:jlo + 2, :],
                                               Ts[jlo][:, 3, :, :], 1.0,
                                               th[:], ALU.add, ALU.mult)

            Ts = {0: None, 2: None}
            # 2-bank pair tiles, 4 in flight: two chunks of PE runway so the
            # ACT read latency never gaps the PE (keeps HAM warm). x-matmuls
            # of chunks 0/1 are H-independent and absorb the previous step's
            # tail; the tail chain runs at chunk-pair granularity and lags
            # so it never head-of-line blocks the gate ACTs.
            xmms(0, 0); xmms(0, 1); xmms(1, 0); xmms(1, 1)
            dr(0, 0, 0, 2); dr(0, 0, 2, 4); dr(0, 1, 0, 2); dr(0, 1, 2, 4)
            acts(0)
            dr(1, 0, 0, 2); dr(1, 0, 2, 4); dr(1, 1, 0, 2); dr(1, 1, 2, 4)
            acts(1)
            xmms(2, 0); xmms(2, 1)
            dr(2, 0, 0, 2); dr(2, 0, 2, 4); dr(2, 1, 0, 2); dr(2, 1, 2, 4)
            acts(2)
            chainP(0)
            thH(0)
            xmms(3, 0); xmms(3, 1)
            dr(3, 0, 0, 2); dr(3, 0, 2, 4); dr(3, 1, 0, 2); dr(3, 1, 2, 4)
            acts(3)
            chainP(2)
            thH(2)
            return Hn8, Sn

        # ======== Phase 2: encoder ========
        Hp8 = spool.tile([128, HK, B], f8, tag="H", name="H")
        Sp = spool.tile([128, HK, B], bf16d, tag="S", name="S")
        nc.vector.memset(Hp8[:], 0.2)
        nc.vector.memset(Sp[:], 0.2)
        with tc.tile_pool(name="gpse", bufs=4, space="PSUM") as gps_e:
            for s in range(L):
                Hp8, Sp = lstm_step(gps_e, xenc, s, Hp8, Sp)
        Henc = Hp8

        # ======== Phase 3: t path + attention ========
        with ExitStack() as ph:
            asb = ph.enter_context(tc.tile_pool(name="asb", bufs=2))
            aps = ph.enter_context(tc.tile_pool(name="aps", bufs=1, space="PSUM"))
            sps_pool = ph.enter_context(tc.tile_pool(name="sps", bufs=2, space="PSUM"))
            vstr = ph.enter_context(tc.tile_pool(name="vstr", bufs=3))

            # t = relu(Wenc' @ Henc + benc), column-normalized
            tr = []
            s_ps = aps.tile([1, B], f32, tag="tsum", name="tsum")
            for mi in range(HK):
                t_ps = sps_pool.tile([128, B], f32, tag="sims", name="sims")
                nc.tensor.matmul(t_ps[:], Wenc8[:, 0:2, mi * 128:(mi + 1) * 128],
                                 Henc[:, 0:2, :], start=True, stop=False,
                                 perf_mode=DR)
                nc.tensor.matmul(t_ps[:], Wenc8[:, 2:4, mi * 128:(mi + 1) * 128],
                                 Henc[:, 2:4, :], start=False, stop=True,
                                 perf_mode=DR)
                tr_mi = asb.tile([128, B], f32, tag=f"tr{mi}", name=f"tr{mi}", bufs=1)
                nc.scalar.activation(tr_mi[:], t_ps[:], AF.Relu, scale=1.0 / SF,
                                     bias=benc[:, mi:mi + 1])
                tr.append(tr_mi)
                sq = asb.tile([128, B], f32r, tag="tsq", name="tsq")
                nc.scalar.activation(sq[:], tr_mi[:], AF.Square)
                nc.tensor.matmul(s_ps[:], ones_col[:], sq[:],
                                 start=(mi == 0), stop=(mi == HK - 1))
            lnr = asb.tile([1, B], f32, tag="tlnr", name="tlnr")
            nc.scalar.activation(lnr[:], s_ps[:], AF.Ln)
            rs_r = asb.tile([1, B], f32r, tag="trs", name="trs")
            nc.scalar.activation(rs_r[:], lnr[:], AF.Exp, scale=-0.5,
                                 bias=ln16_t[:])
            bc_ps = aps.tile([128, B], f32, tag="tbc", name="tbc")
            nc.tensor.matmul(bc_ps[:], ones_row[:], rs_r[:], start=True, stop=True)
            tnp = [asb.tile([128, 2, B], f8, tag=f"tnp{g}", name=f"tnp{g}", bufs=1)
                   for g in range(2)]
            for mi in range(HK):
                nc.vector.tensor_tensor(tnp[mi // 2][:, mi % 2, :], tr[mi][:],
                                        bc_ps[:], ALU.mult)

            # attention: stream gathered vn pairs, E = 16*exp(sims) in fp8,
            # accumulate h with DoubleRow pairs over batch chunks
            hu_ps = [aps.tile([128, B], f32, tag=f"hu{h}", name=f"hu{h}") for h in range(HK)]
            for ip in range(VCHUNKS // 2):
                vnp_i = vstr.tile([128, 2, H], f8, tag="vni", name="vni", bufs=4)
                Ep = vstr.tile([128, 2, B], f8, tag="E", name="E")
                for t_ in range(2):
                    i = 2 * ip + t_
                    r, b = divmod(i, BK)
                    nc.sync.dma_start(vnp_i[:, t_, :], ag_out[r, b])
                    sim_ps = sps_pool.tile([128, B], f32, tag="sims", name="sims")
                    for g in range(2):
                        nc.tensor.matmul(sim_ps[:],
                                         vnTp[g][:, :, i * 128:(i + 1) * 128],
                                         tnp[g][:], start=(g == 0),
                                         stop=(g == 1), perf_mode=DR)
                    nc.scalar.activation(Ep[:, t_, :], sim_ps[:], AF.Exp,
                                         scale=1.0 / 256, bias=ln16c[:])
                for h in range(HK):
                    nc.tensor.matmul(hu_ps[h][:],
                                     vnp_i[:, :, h * 128:(h + 1) * 128], Ep[:],
                                     start=(ip == 0),
                                     stop=(ip == VCHUNKS // 2 - 1),
                                     perf_mode=DR)
            # normalize h (x2 for the doubled-state convention) -> decoder init
            s2_ps = aps.tile([1, B], f32, tag="tsum", name="tsum")
            for h in range(HK):
                sq = asb.tile([128, B], f32r, tag="husq", name="husq")
                nc.scalar.activation(sq[:], hu_ps[h][:], AF.Square)
                nc.tensor.matmul(s2_ps[:], ones_col[:], sq[:],
                                 start=(h == 0), stop=(h == HK - 1))
            lnr2 = asb.tile([1, B], f32, tag="hulnr", name="hulnr")
            nc.scalar.activation(lnr2[:], s2_ps[:], AF.Ln)
            rs2 = asb.tile([1, B], f32r, tag="hurs", name="hurs")
            nc.scalar.activation(rs2[:], lnr2[:], AF.Exp, scale=-0.5, bias=ln2_t[:])
            bc2_ps = aps.tile([128, B], f32, tag="tbc", name="tbc")
            nc.tensor.matmul(bc2_ps[:], ones_row[:], rs2[:], start=True, stop=True)
            bc2_sb = asb.tile([128, B], f32, tag="bc2sb", name="bc2sb", bufs=1)
            nc.vector.tensor_copy(bc2_sb[:], bc2_ps[:])
            H0 = spool.tile([128, HK, B], f8, tag="H", name="H")
            S0 = spool.tile([128, HK, B], bf16d, tag="S", name="S")
            for h in range(HK):
                nc.vector.tensor_tensor(H0[:, h, :], hu_ps[h][:], bc2_sb[:], ALU.mult)
                nc.vector.tensor_tensor(S0[:, h, :], hu_ps[h][:], bc2_sb[:], ALU.mult)

        # ======== Phase 4: decoder ========
        dsb = top.enter_context(tc.tile_pool(name="dsb", bufs=2))
        with tc.tile_pool(name="gpsd", bufs=4, space="PSUM") as gps_d:
            Hp8, Sp = H0, S0
            for s in range(L):
                # prefetch the step's one-hot target mask (gpsimd queue)
                oh = dsb.tile([128, B], f32r, tag="oh", name="oh")
                nc.gpsimd.dma_start(oh[:], oneh_d[s])
                Hp8, Sp = lstm_step(gps_d, xdec, s, Hp8, Sp)
                # logitsT [V, B] in bank 0 of a rotating pair tile; Z and
                # target exp(logit) reductions land in bank 1 (partitions 0/32)
                lq = gps_d.tile([128, 2, B], f32, tag="pair", name="pair")
                nc.tensor.matmul(lq[:, 0, :], Wout8[:, 0:2, :], Hp8[:, 0:2, :],
                                 start=True, stop=False, perf_mode=DR)
                nc.tensor.matmul(lq[:, 0, :], Wout8[:, 2:4, :], Hp8[:, 2:4, :],
                                 start=False, stop=True, perf_mode=DR)
                El = dsb.tile([128, B], f32r, tag="El", name="El")
                nc.scalar.activation(El[:], lq[:, 0, :], AF.Exp, scale=1.0 / SF)
                nc.tensor.matmul(lq[0:1, 1, :], ones_col[:], El[:],
                                 start=True, stop=True)
                ztmp = dsb.tile([1, B], f32, tag="ztmp", name="ztmp")
                nc.vector.tensor_copy(ztmp[:], lq[0:1, 1, :])
                nc.sync.dma_start(Zs[s:s + 1, :], ztmp[:])
                # exp(target logit) via host one-hot * El (on the otherwise
                # idle gpsimd); ln() undoes the exp in the post-pass
                mk = dsb.tile([128, B], f32r, tag="mk", name="mk")
                nc.gpsimd.tensor_tensor(mk[:], oh[:], El[:], ALU.mult)
                nc.tensor.matmul(lq[0:1, 1, :], ones_col[:], mk[:],
                                 start=True, stop=True)
                lttmp = dsb.tile([1, B], f32r, tag="lttmp", name="lttmp")
                nc.vector.tensor_copy(lttmp[:], lq[0:1, 1, :])
                nc.sync.dma_start(lts[s:s + 1, :], lttmp[:])

            # ======== Phase 5: final loss ========
            lnZ = dsb.tile([L, B], f32r, tag="lnZ", name="lnZ")
            nc.scalar.activation(lnZ[:], Zs[:], AF.Ln)
            lnLt = dsb.tile([L, B], f32r, tag="lnLt", name="lnLt")
            nc.scalar.activation(lnLt[:], lts[:], AF.Ln)
            diff = dsb.tile([L, B], f32r, tag="diff", name="diff")
            nc.vector.tensor_tensor(diff[:], lnZ[:], lnLt[:], ALU.subtract)
            fq = gps_d.tile([128, 2, B], f32, tag="pair", name="pair")
            nc.tensor.matmul(fq[0:1, 0, :], ones16[:], diff[:], start=True, stop=True)
            loss_sb = dsb.tile([1, B], f32, tag="losssb", name="losssb")
            nc.vector.tensor_scalar(loss_sb[:], fq[0:1, 0, :], 1.0 / L, None,
                                    ALU.mult)
            nc.sync.dma_start(out_d, loss_sb[:])

    nc.compile()
    return nc


def _prep_inputs(visual_input, text_input, emb, W_ih, W_hh, b_ih, b_hh,
                 W_enc, b_enc, W_out, W_vis):
    import ml_dtypes
    bf = ml_dtypes.bfloat16
    f8n = ml_dtypes.float8_e4m3
    f = np.float32
    vis = np.asarray(visual_input, f)[:, 0, :]              # [4096, 2048]
    text = np.asarray(text_input)
    emb = np.asarray(emb, f)
    visT = np.ascontiguousarray(vis.T)                      # [2048, 4096]
    WvisT = (SF * np.asarray(W_vis, f)).T                   # [2048, 512]
    Wvis8 = np.ascontiguousarray(
        WvisT.reshape(VISK // 2, 2, 128, H).transpose(0, 2, 1, 3)).astype(f8n)

    # x-part weights x64 with bias folded as row 50; g-gate block x2 so all
    # gates share the ACT scale 0.5/64
    b = np.asarray(b_ih, f) + np.asarray(b_hh, f)           # [2048]
    WihT = np.concatenate([np.asarray(W_ih, f).T, b[None, :]], axis=0) * SF
    WihT[:, 2 * H:3 * H] *= 2.0                             # [51, 2048]

    def pack8(WT):                                          # [512, M] -> [128, 4, M]
        M = WT.shape[1]
        return np.ascontiguousarray(
            WT.reshape(HK, 128, M).transpose(1, 0, 2)).astype(f8n)

    WhhT = (0.5 * SF) * np.asarray(W_hh, f).T               # [512, 2048]
    WhhT[:, 2 * H:3 * H] *= 2.0
    Whh8 = pack8(WhhT)
    Wenc8 = pack8((0.5 * SF) * np.asarray(W_enc, f).T)      # [512, 512]
    Wout8 = pack8((0.5 * SF) * np.asarray(W_out, f).T)      # [512, 128]
    benc = np.ascontiguousarray(np.asarray(b_enc, f).reshape(HK, 128).T)

    encx = emb[text.T]                                      # [16, 4096, 50]
    dec_ch = np.concatenate([np.zeros((text.shape[0], 1), text.dtype),
                             text[:, :-1]], axis=1)
    decx = emb[dec_ch.T]                                    # [16, 4096, 50]
    one_row = np.ones((L, 1, B_FULL), f)
    encxT = np.concatenate([encx.transpose(0, 2, 1), one_row], axis=1)
    decxT = np.concatenate([decx.transpose(0, 2, 1), one_row], axis=1)
    encxT = np.ascontiguousarray(encxT)                     # [16, 51, 4096]
    decxT = np.ascontiguousarray(decxT)
    tgt = np.ascontiguousarray(text.T.astype(f))            # [16, 4096]
    iota = np.arange(128, dtype=f).reshape(128, 1)
    # one-hot target masks [L, V, B_FULL]
    oneh = (tgt[:, None, :] == iota.reshape(1, 128, 1)).astype(f)

    in_maps = []
    for c in range(NCORES):
        sl = slice(c * B, (c + 1) * B)
        in_maps.append({
            "visT": np.ascontiguousarray(
                visT[:, sl].reshape(VISK // 2, 2, 128, B).transpose(0, 2, 1, 3)
            ).astype(f8n),
            "WvisT": Wvis8,
            "WihT": WihT.astype(bf),
            "Whh8": Whh8,
            "encx": np.ascontiguousarray(encxT[:, :, sl]).astype(bf),
            "decx": np.ascontiguousarray(decxT[:, :, sl]).astype(bf),
            "Wenc8": Wenc8,
            "benc": benc,
            "Wout8": Wout8,
            "tgt": np.ascontiguousarray(tgt[:, sl]),
            "iota128": iota,
            "oneh": np.ascontiguousarray(oneh[:, :, sl]),
            "ones128": np.ones((128, 1), np.float32),
        })
    return in_maps


LAST_EXEC_TIME_NS = None


def kernel(**inputs):
    global LAST_EXEC_TIME_NS
    from concourse.bass_utils import run_bass_kernel_spmd

    if "nc" not in _CACHE:
        _CACHE["nc"] = _build()
    nc = _CACHE["nc"]
    in_maps = _prep_inputs(**inputs)

    trace = bool(int(os.environ.get("KERNEL_PROFILE", "0")))
    kw = {}
    if trace:
        _install_profile_hook()
        kw["trace"] = True
    res = run_bass_kernel_spmd(nc, in_maps, core_ids=list(range(NCORES)), **kw)
    LAST_EXEC_TIME_NS = res.exec_time_ns
    out = np.concatenate([res.results[c]["loss"][0] for c in range(NCORES)])
    return out.astype(np.float32)


def _install_profile_hook():
    """Optional NTFF profiling (dev only; used when KERNEL_PROFILE=1)."""
    import types, ctypes, contextlib
    try:
        import antenv
    except ImportError:
        return
    if getattr(antenv, "axon_hooks", None) is not None:
        return
    mod = types.ModuleType('antenv.axon_hooks')
    _store = [None]
    mod.set_axon_ntff_profile_hook = lambda h: _store.__setitem__(0, h)
    mod.get_axon_ntff_profile_hook = lambda: _store[0]
    sys.modules['antenv.axon_hooks'] = mod
    antenv.axon_hooks = mod
    try:
        lib = ctypes.CDLL('/opt/axon/libaxon_pjrt.so')
    except OSError:
        return
    if not hasattr(lib, 'axon_start_nrt_profile'):
        return
    lib.axon_start_nrt_profile.argtypes = [ctypes.POINTER(ctypes.c_int64),
                                           ctypes.c_size_t]
    lib.axon_start_nrt_profile.restype = ctypes.c_int64
    lib.axon_stop_nrt_profile.argtypes = [ctypes.c_char_p]
    lib.axon_stop_nrt_profile.restype = ctypes.c_int64

    @contextlib.contextmanager
    def _hook(output_dir, device_ids):
        import jax
        jax.devices()
        if device_ids:
            ids = (ctypes.c_int64 * len(device_ids))(*device_ids)
            rc = lib.axon_start_nrt_profile(ids, len(device_ids))
        else:
            rc = lib.axon_start_nrt_profile(None, 0)
        if rc != 0:
            raise RuntimeError(f"axon_start_nrt_profile rc={rc}")
        try:
            yield
        finally:
            n = lib.axon_stop_nrt_profile(str(output_dir).encode())
            print(f"profile: {n} ntff file(s) in {output_dir}", file=sys.stderr)

    mod.set_axon_ntff_profile_hook = mod.set_axon_ntff_profile_hook
    mod.set_axon_ntff_profile_hook(_hook)
    import concourse.bass_utils as bu
    bu.upload_artifacts = lambda tmpdir: "local://" + str(tmpdir)
